# revision 14
# baseline (speedup 1.0000x reference)
"""Trainium2 Bass kernel for nn_CrAKN (dense transformer with pairwise bias chain).

Sharding: rows of the N=512 crystal dimension are split across 8 cores
(64 rows each). Each core computes its [64, N, 512] bias-chain slice and its
64 attention rows; per layer the updated residual rows are AllGathered so
every core can form the full k/v for the next layer.

Self-contained: hardcodes all shapes; builds one SPMD Bass program and runs
it via run_bass_kernel_spmd on cores 0-7.
"""

import os
import sys
import functools
from contextlib import ExitStack

import numpy as np

sys.path.insert(0, "/opt/trn_rl_repo")

import concourse.bass as bass  # noqa: E402
import concourse.bacc as bacc  # noqa: E402
import concourse.tile as tile  # noqa: E402
import concourse.mybir as mybir  # noqa: E402
import concourse.bass_utils as bass_utils  # noqa: E402
from concourse.masks import make_identity  # noqa: E402
from concourse.dve_ops import AFFINE_MUL_REDUCE  # noqa: E402

F32 = mybir.dt.float32
BF16 = mybir.dt.bfloat16
NP_BF16 = mybir.dt.np(BF16)

AF = mybir.ActivationFunctionType
ALU = mybir.AluOpType
AX = mybir.AxisListType

N, FB, D, H, HD, L, K = 512, 256, 64, 128, 4, 4, 100
H, HD = 4, 128
HHD = H * HD  # 512
NCORES = 8
R = N // NCORES  # 64 rows per core
EPS = 1e-5
SCALE = 1.0 / float(np.sqrt(HD))


def _ln_tiles(nc, tc, pools, in_ap, parts, g_ap, b_ap, out_ap):
    """LayerNorm along the free dim (D=64) of in_ap [parts, 64] -> out_ap."""
    stat = pools["stat"]
    work = pools["work64"]
    ssum = stat.tile([parts, 1], F32, tag="ln_sum")
    nc.vector.reduce_sum(ssum[:], in_ap, axis=AX.X)
    mu = stat.tile([parts, 1], F32, tag="ln_mu")
    nc.vector.tensor_scalar(mu[:], ssum[:], 1.0 / D, None, ALU.mult)
    cen = work.tile([parts, D], F32, tag="ln_cen")
    nc.vector.tensor_scalar(cen[:], in_ap, mu[:], None, ALU.subtract)
    var = stat.tile([parts, 1], F32, tag="ln_var")
    vscr = work.tile([parts, D], F32, tag="ln_xg")
    nc.vector.tensor_tensor(vscr[:], cen[:], cen[:], ALU.mult)
    nc.vector.reduce_sum(var[:], vscr[:], axis=AX.X)
    sd = stat.tile([parts, 1], F32, tag="ln_sd")
    nc.scalar.activation(sd[:], var[:], AF.Sqrt, scale=1.0 / D,
                         bias=pools["eps"][0:parts, :])
    rs = stat.tile([parts, 1], F32, tag="ln_rs")
    nc.vector.reciprocal(rs[:], sd[:])
    xn = work.tile([parts, D], F32, tag="ln_xn")
    nc.vector.tensor_scalar(xn[:], cen[:], rs[:], None, ALU.mult)
    xg = work.tile([parts, D], F32, tag="ln_xg")
    nc.vector.tensor_tensor(xg[:], xn[:], g_ap, ALU.mult)
    nc.vector.tensor_tensor(out_ap, xg[:], b_ap, ALU.add)


@functools.lru_cache(maxsize=4)
def _build(diffb_nonzero: bool, trunc: int = 0):
    nc = bacc.Bacc("TRN2", target_bir_lowering=False, debug=False,
                   enable_asserts=False, num_devices=NCORES)

    def din(name, shape, dt=F32):
        return nc.dram_tensor(name, list(shape), dt, kind="ExternalInput").ap()

    nfT_aug = din("nfT_aug", (FB + 1, N))
    nfT_loc = din("nfT_loc", (FB + 1, R))
    amdsT_aug = din("amdsT_aug", (K + 1, N))
    amdsT_loc = din("amdsT_loc", (K + 1, R))
    embW_aug = din("embW_aug", (FB + 1, D))
    bembW_aug = din("bembW_aug", (K + 1, D))
    qkvW_aug_d = din("qkvW_aug", (L, D + 1, 3 * HHD), BF16)
    dWf0_aug_d = din("dWf0_aug", (D + 1, HHD))
    diffW_dup_d = din("diffW_dup", (L, 2 * D, HHD), BF16)
    diffb_d = din("diffb_cols", (L, HD, H))
    boutW_dup_d = din("boutW_dup", (L, HD, 8 * D), BF16)
    boutb_d = din("boutb2", (HD, L))
    oW_d = din("oW", (L, HHD, D), BF16)
    ob_d = din("ob_cols", (D, L))
    outW_aug_d = din("outW_aug", (D + 1, 1))
    ln1g_d = din("ln1g_t", (HD, D))
    ln1b_d = din("ln1b_t", (HD, D))
    ln2g_d = din("ln2g_t", (HD, D))
    ln2b_d = din("ln2b_t", (HD, D))
    strip_d = din("strip", (HD, 255), BF16)

    out_dram = nc.dram_tensor("out_loc", [R, 1], F32, kind="ExternalOutput").ap()

    with nc.allow_low_precision(reason="bf16 mish rational chain"), \
         tile.TileContext(nc) as tc, ExitStack() as ctx:
        cpool = ctx.enter_context(tc.tile_pool(name="const", bufs=1))
        ppool = ctx.enter_context(tc.tile_pool(name="persist", bufs=1))
        wpool = ctx.enter_context(tc.tile_pool(name="work", bufs=2))
        w2pool = ctx.enter_context(tc.tile_pool(name="work2", bufs=2))
        w64 = ctx.enter_context(tc.tile_pool(name="work64", bufs=2))
        statp = ctx.enter_context(tc.tile_pool(name="stat", bufs=4))
        ps_be = ctx.enter_context(tc.tile_pool(name="ps_be", bufs=2, space="PSUM"))
        ps_d = ctx.enter_context(tc.tile_pool(name="ps_d", bufs=1, space="PSUM"))
        ps_bn = ctx.enter_context(tc.tile_pool(name="ps_bn", bufs=1, space="PSUM"))
        ps_x = ctx.enter_context(tc.tile_pool(name="ps_x", bufs=1, space="PSUM"))
        dram = ctx.enter_context(tc.tile_pool(name="dram", bufs=1, space="DRAM"))
        pools = {"stat": statp, "work64": w64}

        dma = nc.sync.dma_start

        # ---------------- constants into SBUF ----------------
        def cload(name, shape, src_ap, dt=F32):
            t = cpool.tile(list(shape), dt, tag=name, name=name)
            dma(t[:], src_ap)
            return t

        # node features transposed (3 K-chunks: 128/128/1)
        nfT0 = cload("nfT0", [128, N], nfT_aug[0:128, :])
        nfT1 = cload("nfT1", [128, N], nfT_aug[128:256, :])
        nfT2 = cload("nfT2", [1, N], nfT_aug[256:257, :])
        nfl0 = cload("nfl0", [128, R], nfT_loc[0:128, :])
        nfl1 = cload("nfl1", [128, R], nfT_loc[128:256, :])
        nfl2 = cload("nfl2", [1, R], nfT_loc[256:257, :])
        embW0 = cload("embW0", [128, D], embW_aug[0:128, :])
        embW1 = cload("embW1", [128, D], embW_aug[128:256, :])
        embW2 = cload("embW2", [1, D], embW_aug[256:257, :])
        amds_sb = cload("amds_sb", [K + 1, N], amdsT_aug[:, :])
        amdl_sb = cload("amdl_sb", [K + 1, R], amdsT_loc[:, :])
        bembW = cload("bembW", [K + 1, D], bembW_aug[:, :])
        dWf0 = cload("dWf0", [D + 1, HHD], dWf0_aug_d[:, :])
        qkvW = [cload(f"qkvW{l}", [D + 1, 3 * HHD], qkvW_aug_d[l, :, :], BF16)
                for l in range(L)]
        diffW = [cload(f"diffW{l}", [2 * D, HHD], diffW_dup_d[l, :, :], BF16)
                 for l in range(1, L)]
        diffW = [None] + diffW
        diffb = [cload(f"diffb{l}", [HD, H], diffb_d[l, :, :])
                 for l in range(L)] if diffb_nonzero else None
        boutW = [cload(f"boutW{l}", [HD, 8 * D], boutW_dup_d[l, :, :], BF16)
                 for l in range(L - 1)]
        boutb = cload("boutb", [HD, L], boutb_d[:, :])
        oW_sb = []
        for l in range(L):
            t = cpool.tile([HD, H * D], BF16, tag=f"oW{l}", name=f"oW{l}")
            for h in range(H):
                dma(t[:, h * D:(h + 1) * D], oW_d[l, h * HD:(h + 1) * HD, :])
            oW_sb.append(t)
        ob_sb = cload("ob_sb", [D, L], ob_d[:, :])
        outW_sb = cload("outW_sb", [D + 1, 1], outW_aug_d[:, :])
        ln1g = cload("ln1g", [HD, D], ln1g_d[:, :])
        ln1b = cload("ln1b", [HD, D], ln1b_d[:, :])
        ln2g = cload("ln2g", [HD, D], ln2g_d[:, :])
        ln2b = cload("ln2b", [HD, D], ln2b_d[:, :])
        strip = cload("strip", [HD, 255], strip_d[:, :], BF16)

        ident = cpool.tile([128, 128], F32, tag="ident")
        make_identity(nc, ident[:])
        identb = cpool.tile([128, 128], BF16, tag="identb")
        make_identity(nc, identb[:])
        epsc = cpool.tile([128, 1], F32, tag="epsc")
        nc.gpsimd.memset(epsc[:], EPS)
        pools["eps"] = epsc
        onec = cpool.tile([128, 1], F32, tag="onec")
        nc.gpsimd.memset(onec[:], 1.0)

        # ---------------- persistent tiles ----------------
        biasA = ppool.tile([128, R * HHD // 2], BF16, tag="biasA")
        biasB = ppool.tile([128, R * HHD // 2], BF16, tag="biasB")
        b0T = ppool.tile([D + 1, N], F32, tag="b0T")
        b0L = ppool.tile([D, R], F32, tag="b0L")
        Gp = ppool.tile([128, H * N], BF16, tag="Gp")
        nGl = ppool.tile([128, H * R], F32, tag="Gl")
        xT = ppool.tile([D + 1, N], BF16, tag="xT")
        xlocT = ppool.tile([D + 1, R], BF16, tag="xlocT")
        x_loc = ppool.tile([R, D], F32, tag="x_loc")
        resid_loc = ppool.tile([R, D], F32, tag="resid_loc")
        pre_all = ppool.tile([128, 4 * D], F32, tag="pre_all")
        xfull = ppool.tile([128, 4 * D], F32, tag="xfull")
        kT = ppool.tile([HD, H * N], BF16, tag="kT")
        v_all = ppool.tile([128, H * HD * 4 // 4 * 4], BF16, tag="v_all")  # [128, 2048]
        ql = ppool.tile([HD, H * R], BF16, tag="ql")
        va = ppool.tile([HD, H * R], BF16, tag="va")
        diffs_s = [ppool.tile([128, N], F32, tag=f"diffs{p}", name=f"diffs{p}")
                   for p in range(2)]
        xfT = ppool.tile([D + 1, R], F32, tag="xfT")

        # collective bounce buffers
        gin = [dram.tile([R, D], F32, tag=f"gin{l}", name=f"gin{l}")
               for l in range(L - 1)]
        gout = [dram.tile([N, D], F32, tag=f"gout{l}", name=f"gout{l}")
                for l in range(L - 1)]

        # ---------------- head: h, b0, G ----------------
        # full pre-activation h rows -> pre_all ([128, 64] x 4 tiles)
        for m in range(4):
            ph = ps_x.tile([128, D], F32, tag="x")
            nc.tensor.matmul(ph[:], nfT0[:, m * 128:(m + 1) * 128], embW0[:],
                             start=True, stop=False)
            nc.tensor.matmul(ph[:], nfT1[:, m * 128:(m + 1) * 128], embW1[:],
                             start=False, stop=False)
            nc.tensor.matmul(ph[:], nfT2[:, m * 128:(m + 1) * 128], embW2[:],
                             start=False, stop=True)
            nc.vector.tensor_copy(out=pre_all[:, m * D:(m + 1) * D], in_=ph[:])
        # local pre-activation rows -> resid_loc
        pl = ps_x.tile([R, D], F32, tag="x")
        nc.tensor.matmul(pl[:], nfl0[:], embW0[:], start=True, stop=False)
        nc.tensor.matmul(pl[:], nfl1[:], embW1[:], start=False, stop=False)
        nc.tensor.matmul(pl[:], nfl2[:], embW2[:], start=False, stop=True)
        nc.vector.tensor_copy(resid_loc[:], pl[:])
        # b0T (augmented with ones row), b0L
        pb = ps_x.tile([D, N], F32, tag="x")
        nc.tensor.matmul(pb[:], bembW[:], amds_sb[:], start=True, stop=True)
        nc.vector.tensor_copy(out=b0T[0:D, :], in_=pb[:])
        nc.gpsimd.memset(b0T[D:D + 1, :], 1.0)
        pbl = ps_x.tile([D, R], F32, tag="x")
        nc.tensor.matmul(pbl[:], bembW[:], amdl_sb[:], start=True, stop=True)
        nc.vector.tensor_copy(b0L[:], pbl[:])
        # G' = b0 @ diff_W0 + diff_b0 (full, via augmented row) -> Gp (bf16)
        # G'' = b0_loc @ diff_W0 (local) -> Gl (f32)
        for m in range(4):
            pg = ps_x.tile([128, N], F32, tag="x")
            nc.tensor.matmul(pg[:], dWf0[:, m * 128:(m + 1) * 128], b0T[:],
                             start=True, stop=True)
            nc.vector.tensor_copy(out=Gp[:, m * N:(m + 1) * N], in_=pg[:])
            pgl = ps_x.tile([128, R], F32, tag="x")
            nc.tensor.matmul(pgl[:], dWf0[0:D, m * 128:(m + 1) * 128], b0L[:],
                             start=True, stop=True)
            nc.vector.tensor_scalar(nGl[:, m * R:(m + 1) * R], pgl[:],
                                    -1.0, None, ALU.mult)

        def _early_out():
            osb_e = w64.tile([R, 1], F32, tag="osb", name="osb_e")
            nc.vector.tensor_copy(osb_e[:], resid_loc[:, 0:1])
            nc.sync.dma_start(out_dram[:, :], osb_e[:])

        if trunc == 1:
            _early_out()
        n_layers = L if trunc == 0 else min(L, trunc - 1)

        # ---------------- layers ----------------
        for l in range(n_layers):
            bias_cur = biasA if l in (1, 3) else biasB
            bias_nxt = biasA if l == 0 else biasB if l == 1 else biasA

            # ---- (a) i-loop: bias chain ----
            # mish(x) = x*(1 - 2r), r = 1/(u^2+2u+2), u = e^x.  r is computed
            # as exp(-ln(w+2)) on the scalar LUT (exp+ln live in one table),
            # the final multiply as one AFFINE_MUL_REDUCE custom-DVE op.
            # Processed in half tiles [128, 2N] (head pairs) so the be-psum
            # can double-buffer (2 bufs x 2 banks).
            psum_bn = None
            psum_diff = [ps_d.tile([128, N], F32, tag=f"d{q}", name=f"pd{l}_{q}")
                         for q in range(2)]
            for i in range(R):
                half = (i % 2) * D
                for s in range(2):
                    if l == 0:
                        xb = wpool.tile([128, 2 * N], BF16, tag="xb",
                                        name=f"xb{l}_{i}_{s}")
                        for mm in range(2):
                            m = 2 * s + mm
                            nc.vector.tensor_scalar(
                                xb[:, mm * N:(mm + 1) * N],
                                Gp[:, m * N:(m + 1) * N],
                                nGl[:, m * R + i:m * R + i + 1],
                                None, ALU.add)
                        u_t = wpool.tile([128, 2 * N], BF16, tag="u",
                                         name=f"u{l}_{i}_{s}")
                        nc.scalar.activation(u_t[:], xb[:], AF.Exp)
                        x_src = xb
                    else:
                        psum_be = ps_be.tile([128, 2 * N], F32, tag="be")
                        for mm in range(2):
                            m = 2 * s + mm
                            nc.tensor.matmul(
                                psum_be[:, mm * N:(mm + 1) * N],
                                diffW[l][half:half + D, m * 128:(m + 1) * 128],
                                bias_cur[half:half + D,
                                         (i // 2) * HHD:(i // 2) * HHD + HHD],
                                start=True, stop=True)
                        u_t = wpool.tile([128, 2 * N], BF16, tag="u",
                                         name=f"u{l}_{i}_{s}")
                        if diffb_nonzero:
                            xb = wpool.tile([128, 2 * N], BF16, tag="xb",
                                            name=f"xb{l}_{i}_{s}")
                            for mm in range(2):
                                m = 2 * s + mm
                                sl = slice(mm * N, (mm + 1) * N)
                                nc.scalar.activation(xb[:, sl], psum_be[:, sl],
                                                     AF.Identity,
                                                     bias=diffb[l][:, m:m + 1])
                            nc.scalar.activation(u_t[:], xb[:], AF.Exp)
                            x_src = xb
                        else:
                            nc.scalar.activation(u_t[:], psum_be[:], AF.Exp)
                            x_src = psum_be
                    # p = (u+1)^2 ; d = p+1 = u^2+2u+2 ; r ~= 1/d ;
                    # mish = (r*(-2)+1) * x   (one custom-DVE op)
                    p_t = wpool.tile([128, 2 * N], F32, tag="p",
                                     name=f"p{l}_{i}_{s}")
                    nc.scalar.activation(p_t[:], u_t[:], AF.Square,
                                         bias=onec[:])
                    d_t = wpool.tile([128, 2 * N], F32, tag="d",
                                     name=f"d{l}_{i}_{s}")
                    nc.vector.tensor_scalar(d_t[:], p_t[:], 1.0, None, ALU.add)
                    r_t = wpool.tile([128, 2 * N], F32, tag="r",
                                     name=f"r{l}_{i}_{s}")
                    nc.vector.reciprocal_approx_fast(out=r_t[:], in_=d_t[:])
                    mish_t = wpool.tile([128, 2 * N], BF16, tag="mish",
                                        name=f"mish{l}_{i}_{s}")
                    nc.vector._custom_dve(
                        AFFINE_MUL_REDUCE, out=mish_t[:], in0=r_t[:],
                        in1=x_src[:], s0=-2.0, s1=1.0)
                    sq_t = wpool.tile([128, 2 * N], BF16, tag="sq",
                                      name=f"sq{l}_{i}_{s}")
                    nc.gpsimd.tensor_tensor(sq_t[:], mish_t[:], mish_t[:],
                                            ALU.mult)
                    # diffs accumulation (one-hot column matmuls): half s
                    # feeds head pair p == s
                    for hh in range(2):
                        col = hh * D + i
                        nc.tensor.matmul(
                            psum_diff[s][:],
                            strip[:, 127 - col:255 - col],
                            sq_t[:, hh * N:(hh + 1) * N],
                            start=(i == 0 and hh == 0),
                            stop=(i == R - 1 and hh == 1),
                            skip_group_check=True)
                    # next-layer bias (skip on last layer)
                    if l < L - 1:
                        if s == 0 and i % 2 == 0:
                            psum_bn = ps_bn.tile([128, HHD], F32, tag="bn",
                                                 name=f"bn{l}_{i}")
                        for mm in range(2):
                            m = 2 * s + mm
                            nc.tensor.matmul(
                                psum_bn[half:half + D, :],
                                boutW[l][:, m * 128 + half:m * 128 + half + D],
                                mish_t[:, mm * N:(mm + 1) * N],
                                start=(m == 0), stop=(m == 3),
                                tile_position=(0, half))
                if l < L - 1 and i % 2 == 1:
                    # mish on the accumulated [128, HHD] bias tile
                    bsl = slice((i // 2) * HHD, (i // 2) * HHD + HHD)
                    u2 = w2pool.tile([128, HHD], BF16, tag="u2",
                                     name=f"u2_{l}_{i}")
                    nc.scalar.activation(u2[:], psum_bn[:], AF.Exp,
                                         bias=boutb[:, l:l + 1])
                    p2 = w2pool.tile([128, HHD], F32, tag="p2",
                                     name=f"p2_{l}_{i}")
                    nc.scalar.activation(p2[:], u2[:], AF.Square,
                                         bias=onec[:])
                    d2 = w2pool.tile([128, HHD], F32, tag="d2",
                                     name=f"d2_{l}_{i}")
                    nc.vector.tensor_scalar(d2[:], p2[:], 1.0, None, ALU.add)
                    r2 = w2pool.tile([128, HHD], F32, tag="r2",
                                     name=f"r2_{l}_{i}")
                    nc.vector.reciprocal_approx_fast(out=r2[:], in_=d2[:])
                    tm2 = w2pool.tile([128, HHD], BF16, tag="tm2",
                                      name=f"tm2_{l}_{i}")
                    nc.vector.tensor_scalar(tm2[:], r2[:], -2.0, 1.0,
                                            ALU.mult, ALU.add)
                    nc.vector._custom_dve(
                        AFFINE_MUL_REDUCE, out=bias_nxt[:, bsl],
                        in0=psum_bn[:], in1=tm2[:], s0=1.0,
                        s1=boutb[:, l:l + 1])

            # ---- (b) sqrt window: diffs sqrt + LN -> x_l ----
            for p in range(2):
                nc.scalar.activation(diffs_s[p][:], psum_diff[p][:], AF.Sqrt)
            if l == n_layers - 1 and trunc != 0 and os.environ.get("KHALF") == "1":
                break
            if l > 0:
                for m in range(4):
                    dma(pre_all[:, m * D:(m + 1) * D],
                        gout[l - 1][m * 128:(m + 1) * 128, :])
            g_t, b_t = (ln1g, ln1b) if l == 0 else (ln2g, ln2b)
            for m in range(4):
                _ln_tiles(nc, tc, pools, pre_all[:, m * D:(m + 1) * D], 128,
                          g_t[:], b_t[:], xfull[:, m * D:(m + 1) * D])
            _ln_tiles(nc, tc, pools, resid_loc[:], R,
                      g_t[0:R, :], b_t[0:R, :], x_loc[:])
            if l == n_layers - 1 and trunc != 0 and \
                    int(os.environ.get("KPHASE", "9")) <= 0:
                break
            # transposes -> xT (augmented), xlocT (augmented)
            for m in range(4):
                pt = ps_x.tile([D, 128], F32, tag="x")
                nc.tensor.transpose(pt[:], xfull[:, m * D:(m + 1) * D], ident[:])
                nc.vector.tensor_copy(out=xT[0:D, m * 128:(m + 1) * 128],
                                      in_=pt[:])
            nc.gpsimd.memset(xT[D:D + 1, :], 1.0)
            ptl = ps_x.tile([D, R], F32, tag="x")
            nc.tensor.transpose(ptl[:], x_loc[:], ident[0:R, 0:R])
            nc.vector.tensor_copy(out=xlocT[0:D, :], in_=ptl[:])
            nc.gpsimd.memset(xlocT[D:D + 1, :], 1.0)
            if l == n_layers - 1 and trunc != 0 and \
                    int(os.environ.get("KPHASE", "9")) <= 1:
                break

            # ---- (c) qkv ----
            for h in range(H):
                base = h * 3 * HD
                # k^T for head h
                pk = ps_x.tile([HD, N], F32, tag="x")
                nc.tensor.matmul(pk[:], qkvW[l][:, base + HD:base + 2 * HD],
                                 xT[:], start=True, stop=True)
                nc.vector.tensor_copy(out=kT[:, h * N:(h + 1) * N], in_=pk[:])
                # q^T local rows
                pq = ps_x.tile([HD, R], F32, tag="x")
                nc.tensor.matmul(pq[:], qkvW[l][:, base:base + HD],
                                 xlocT[:], start=True, stop=True)
                nc.vector.tensor_copy(out=ql[:, h * R:(h + 1) * R], in_=pq[:])
                # v (untransposed) per token chunk
                for tc_ in range(4):
                    pv = ps_x.tile([128, HD], F32, tag="x")
                    nc.tensor.matmul(pv[:], xT[:, tc_ * 128:(tc_ + 1) * 128],
                                     qkvW[l][:, base + 2 * HD:base + 3 * HD],
                                     start=True, stop=True)
                    nc.vector.tensor_copy(
                        out=v_all[:, (h * 4 + tc_) * HD:(h * 4 + tc_ + 1) * HD],
                        in_=pv[:])

            if l == n_layers - 1 and trunc != 0 and \
                    int(os.environ.get("KPHASE", "9")) <= 2:
                break
            # ---- (d) attention per head ----
            for h in range(H):
                p, hh = h // 2, h % 2
                plg = ps_x.tile([R, N], F32, tag="x")
                nc.tensor.matmul(plg[:], ql[:, h * R:(h + 1) * R],
                                 kT[:, h * N:(h + 1) * N], start=True, stop=True)
                pre_sb = wpool.tile([R, N], BF16, tag="pre_sb")
                nc.vector.scalar_tensor_tensor(
                    out=pre_sb[:], in0=plg[:], scalar=SCALE,
                    in1=diffs_s[p][hh * R:(hh + 1) * R, :],
                    op0=ALU.mult, op1=ALU.add)
                nmax = statp.tile([R, 1], F32, tag="nmax")
                nc.vector.reduce_max(nmax[:], pre_sb[:], axis=AX.X, negate=True)
                esb = wpool.tile([R, N], BF16, tag="esb")
                sumexp = statp.tile([R, 1], F32, tag="sumexp")
                nc.scalar.activation(esb[:], pre_sb[:], AF.Exp,
                                     bias=nmax[:], accum_out=sumexp[:])
                rsum = statp.tile([R, 1], F32, tag="rsum")
                nc.vector.reciprocal(rsum[:], sumexp[:])
                att = wpool.tile([R, N], BF16, tag="att")
                nc.vector.tensor_scalar(att[:], esb[:], rsum[:], None, ALU.mult)
                attT = wpool.tile([128, 4 * R], BF16, tag="attT")
                for tc_ in range(4):
                    pat = ps_x.tile([128, R], BF16, tag="x")
                    nc.tensor.transpose(pat[:], att[:, tc_ * 128:(tc_ + 1) * 128],
                                        identb[0:R, 0:R])
                    nc.vector.tensor_copy(out=attT[:, tc_ * R:(tc_ + 1) * R],
                                          in_=pat[:])
                pvl = ps_x.tile([HD, R], F32, tag="x")
                for tc_ in range(4):
                    nc.tensor.matmul(
                        pvl[:],
                        v_all[:, (h * 4 + tc_) * HD:(h * 4 + tc_ + 1) * HD],
                        attT[:, tc_ * R:(tc_ + 1) * R],
                        start=(tc_ == 0), stop=(tc_ == 3))
                nc.vector.tensor_copy(out=va[:, h * R:(h + 1) * R], in_=pvl[:])

            if l == n_layers - 1 and trunc != 0 and \
                    int(os.environ.get("KPHASE", "9")) <= 3:
                break
            # ---- (e) output projection for local rows ----
            ptx = ps_x.tile([D, R], F32, tag="x")
            for h in range(H):
                nc.tensor.matmul(ptx[:], oW_sb[l][:, h * D:(h + 1) * D],
                                 va[:, h * R:(h + 1) * R],
                                 start=(h == 0), stop=(h == 3))
            tempxT = w64.tile([D, R], F32, tag="tempxT")
            nc.scalar.activation(tempxT[:], ptx[:], AF.Identity,
                                 bias=ob_sb[:, l:l + 1])
            # residual: resid_loc = x_loc + temp_x (untransposed)
            ptu = ps_x.tile([R, D], F32, tag="x")
            nc.tensor.transpose(ptu[:], tempxT[:], ident[0:D, 0:D])
            nc.vector.tensor_tensor(resid_loc[:], ptu[:], x_loc[:], ALU.add)

            # ---- (f) gather residual rows (layers 0-2) ----
            if l == n_layers - 1 and trunc != 0 and \
                    int(os.environ.get("KPHASE", "9")) <= 4:
                break
            if l < L - 1:
                nc.sync.dma_start(gin[l][:], resid_loc[:])
                nc.gpsimd.collective_compute(
                    "AllGather", ALU.bypass,
                    replica_groups=[list(range(NCORES))],
                    ins=[gin[l].opt()], outs=[gout[l].opt()])

        # ---------------- final: LN + out head on local rows ----------------
        if trunc > 1:
            _early_out()
        if trunc == 0:
            x4 = w64.tile([R, D], F32, tag="x4")
            _ln_tiles(nc, tc, pools, resid_loc[:], R, ln2g[0:R, :],
                      ln2b[0:R, :], x4[:])
            pxf = ps_x.tile([D, R], F32, tag="x")
            nc.tensor.transpose(pxf[:], x4[:], ident[0:R, 0:R])
            nc.vector.tensor_copy(out=xfT[0:D, :], in_=pxf[:])
            nc.gpsimd.memset(xfT[D:D + 1, :], 1.0)
            pout = ps_x.tile([R, 1], F32, tag="x")
            nc.tensor.matmul(pout[:], xfT[:], outW_sb[:], start=True, stop=True)
            osb = w64.tile([R, 1], F32, tag="osb")
            nc.vector.tensor_copy(osb[:], pout[:])
            nc.sync.dma_start(out_dram[:, :], osb[:])

    nc.compile()
    return nc


def _prep_inputs(inputs):
    f32 = np.float32

    def f(x):
        return np.ascontiguousarray(np.asarray(x), dtype=f32)

    nf = f(inputs["node_features"])
    amds = f(inputs["amds"])
    emb_W, emb_b = f(inputs["emb_W"]), f(inputs["emb_b"])
    bemb_W, bemb_b = f(inputs["bias_emb_W"]), f(inputs["bias_emb_b"])
    qkv_W, qkv_b = f(inputs["qkv_W"]), f(inputs["qkv_b"])
    diff_W, diff_b = f(inputs["diff_W"]), f(inputs["diff_b"])
    o_W, o_b = f(inputs["o_W"]), f(inputs["o_b"])
    bout_W, bout_b = f(inputs["bout_W"]), f(inputs["bout_b"])
    out_W, out_b = f(inputs["out_W"]), f(inputs["out_b"])
    ln1_g, ln1_b = f(inputs["ln1_g"]), f(inputs["ln1_b"])
    ln2_g, ln2_b = f(inputs["ln2_g"]), f(inputs["ln2_b"])

    ones_n = np.ones((1, N), f32)
    ones_r = np.ones((1, R), f32)
    com = {}
    com["nfT_aug"] = np.ascontiguousarray(
        np.concatenate([nf.T, ones_n], 0))
    com["amdsT_aug"] = np.ascontiguousarray(
        np.concatenate([amds.T, ones_n], 0))
    com["embW_aug"] = np.concatenate([emb_W, emb_b[None, :]], 0)
    com["bembW_aug"] = np.concatenate([bemb_W, bemb_b[None, :]], 0)
    com["qkvW_aug"] = np.ascontiguousarray(
        np.concatenate([qkv_W, qkv_b[:, None, :]], 1)).astype(NP_BF16)
    com["dWf0_aug"] = np.concatenate([diff_W[0], diff_b[0][None, :]], 0)
    com["diffW_dup"] = np.ascontiguousarray(
        np.concatenate([diff_W, diff_W], 1)).astype(NP_BF16)
    com["diffb_cols"] = np.ascontiguousarray(
        diff_b.reshape(L, H, HD).transpose(0, 2, 1))
    bwd = np.zeros((L, HD, 8 * D), f32)
    for l in range(L):
        for h in range(H):
            chunk = bout_W[l, h * HD:(h + 1) * HD, :]  # [128, 64]
            bwd[l, :, h * 2 * D:h * 2 * D + D] = chunk
            bwd[l, :, h * 2 * D + D:h * 2 * D + 2 * D] = chunk
    com["boutW_dup"] = bwd.astype(NP_BF16)
    com["boutb2"] = np.ascontiguousarray(
        np.tile(bout_b, (1, 2)).T)  # [128, L]
    com["oW"] = o_W.astype(NP_BF16)
    com["ob_cols"] = np.ascontiguousarray(o_b.T)
    com["outW_aug"] = np.concatenate([out_W, out_b[None, :]], 0)
    com["ln1g_t"] = np.tile(ln1_g[None, :], (HD, 1))
    com["ln1b_t"] = np.tile(ln1_b[None, :], (HD, 1))
    com["ln2g_t"] = np.tile(ln2_g[None, :], (HD, 1))
    com["ln2b_t"] = np.tile(ln2_b[None, :], (HD, 1))
    strip = np.zeros((HD, 255), f32)
    strip[:, 127] = 1.0
    com["strip"] = strip.astype(NP_BF16)

    in_maps = []
    for c in range(NCORES):
        m = dict(com)
        m["nfT_loc"] = np.ascontiguousarray(
            np.concatenate([nf.T[:, c * R:(c + 1) * R], ones_r], 0))
        m["amdsT_loc"] = np.ascontiguousarray(
            np.concatenate([amds.T[:, c * R:(c + 1) * R], ones_r], 0))
        in_maps.append(m)
    diffb_nonzero = bool(np.any(diff_b != 0.0))
    return in_maps, diffb_nonzero


_LAST_RESULTS = None


def kernel(**inputs) -> np.ndarray:
    global _LAST_RESULTS
    in_maps, diffb_nonzero = _prep_inputs(inputs)
    trunc = int(os.environ.get("KTRUNC", "0"))
    nc = _build(diffb_nonzero, trunc)
    trace = bool(int(os.environ.get("KERNEL_TRACE", "0")))
    try:
        res = bass_utils.run_bass_kernel_spmd(
            nc, in_maps, core_ids=list(range(NCORES)), trace=trace)
    except ModuleNotFoundError:
        res = bass_utils.run_bass_kernel_spmd(
            nc, in_maps, core_ids=list(range(NCORES)), trace=False)
    _LAST_RESULTS = res
    out = np.concatenate(
        [res.results[c]["out_loc"] for c in range(NCORES)], axis=0)
    return out.astype(np.float32)


if __name__ == "__main__":
    rng = np.random.default_rng(0)
    dummy = {
        "node_features": rng.standard_normal((N, FB), dtype=np.float32),
        "amds": rng.random((N, K), dtype=np.float32),
        "emb_W": rng.standard_normal((FB, D), dtype=np.float32) / 16,
        "emb_b": np.zeros((D,), np.float32),
        "bias_emb_W": rng.standard_normal((K, D), dtype=np.float32) / 10,
        "bias_emb_b": np.zeros((D,), np.float32),
        "ln1_g": np.ones((D,), np.float32),
        "ln1_b": np.zeros((D,), np.float32),
        "ln2_g": np.ones((D,), np.float32),
        "ln2_b": np.zeros((D,), np.float32),
        "qkv_W": rng.standard_normal((L, D, 3 * HHD), dtype=np.float32) / 8,
        "qkv_b": np.zeros((L, 3 * HHD), np.float32),
        "diff_W": rng.standard_normal((L, D, HHD), dtype=np.float32) / 8,
        "diff_b": np.zeros((L, HHD), np.float32),
        "o_W": rng.standard_normal((L, HHD, D), dtype=np.float32) / 22,
        "o_b": np.zeros((L, D), np.float32),
        "bout_W": rng.standard_normal((L, HHD, D), dtype=np.float32) / 22,
        "bout_b": np.zeros((L, D), np.float32),
        "out_W": rng.standard_normal((D, 1), dtype=np.float32) / 8,
        "out_b": np.zeros((1,), np.float32),
    }
    out = kernel(**dummy)
    print("kernel output shape:", out.shape, "first:", out[:4, 0])



# revision 20
# speedup vs baseline: 1.1741x; 1.1741x over previous
"""Trainium2 Bass kernel for nn_CrAKN (dense transformer with pairwise bias chain).

Sharding: rows of the N=512 crystal dimension are split across 8 cores
(64 rows each). Each core computes its [64, N, 512] bias-chain slice and its
64 attention rows; per layer the updated residual rows are AllGathered so
every core can form the full k/v for the next layer.

Self-contained: hardcodes all shapes; builds one SPMD Bass program and runs
it via run_bass_kernel_spmd on cores 0-7.
"""

import os
import sys
import functools
from contextlib import ExitStack

import numpy as np

sys.path.insert(0, "/opt/trn_rl_repo")

import concourse.bass as bass  # noqa: E402
import concourse.bacc as bacc  # noqa: E402
import concourse.tile as tile  # noqa: E402
import concourse.mybir as mybir  # noqa: E402
import concourse.bass_utils as bass_utils  # noqa: E402
from concourse.masks import make_identity  # noqa: E402
from concourse.dve_ops import AFFINE_MUL_REDUCE  # noqa: E402

F32 = mybir.dt.float32
BF16 = mybir.dt.bfloat16
NP_BF16 = mybir.dt.np(BF16)

AF = mybir.ActivationFunctionType
ALU = mybir.AluOpType
AX = mybir.AxisListType

N, FB, D, H, HD, L, K = 512, 256, 64, 128, 4, 4, 100
H, HD = 4, 128
HHD = H * HD  # 512
NCORES = 8
R = N // NCORES  # 64 rows per core
EPS = 1e-5
SCALE = 1.0 / float(np.sqrt(HD))


def _ln_tiles(nc, tc, pools, in_ap, parts, g_ap, b_ap, out_ap):
    """LayerNorm along the free dim (D=64) of in_ap [parts, 64] -> out_ap."""
    stat = pools["stat"]
    work = pools["work64"]
    ssum = stat.tile([parts, 1], F32, tag="ln_sum")
    nc.vector.reduce_sum(ssum[:], in_ap, axis=AX.X)
    mu = stat.tile([parts, 1], F32, tag="ln_mu")
    nc.vector.tensor_scalar(mu[:], ssum[:], 1.0 / D, None, ALU.mult)
    cen = work.tile([parts, D], F32, tag="ln_cen")
    nc.vector.tensor_scalar(cen[:], in_ap, mu[:], None, ALU.subtract)
    var = stat.tile([parts, 1], F32, tag="ln_var")
    vscr = work.tile([parts, D], F32, tag="ln_xg")
    nc.vector.tensor_tensor(vscr[:], cen[:], cen[:], ALU.mult)
    nc.vector.reduce_sum(var[:], vscr[:], axis=AX.X)
    sd = stat.tile([parts, 1], F32, tag="ln_sd")
    nc.scalar.activation(sd[:], var[:], AF.Sqrt, scale=1.0 / D,
                         bias=pools["eps"][0:parts, :])
    rs = stat.tile([parts, 1], F32, tag="ln_rs")
    nc.vector.reciprocal(rs[:], sd[:])
    xn = work.tile([parts, D], F32, tag="ln_xn")
    nc.vector.tensor_scalar(xn[:], cen[:], rs[:], None, ALU.mult)
    xg = work.tile([parts, D], F32, tag="ln_xg")
    nc.vector.tensor_tensor(xg[:], xn[:], g_ap, ALU.mult)
    nc.vector.tensor_tensor(out_ap, xg[:], b_ap, ALU.add)


@functools.lru_cache(maxsize=4)
def _build(diffb_nonzero: bool, trunc: int = 0):
    nc = bacc.Bacc("TRN2", target_bir_lowering=False, debug=False,
                   enable_asserts=False, num_devices=NCORES)

    def din(name, shape, dt=F32):
        return nc.dram_tensor(name, list(shape), dt, kind="ExternalInput").ap()

    nfT_aug = din("nfT_aug", (FB + 1, N))
    nfT_loc = din("nfT_loc", (FB + 1, R))
    amdsT_aug = din("amdsT_aug", (K + 1, N))
    amdsT_loc = din("amdsT_loc", (K + 1, R))
    embW_aug = din("embW_aug", (FB + 1, D))
    bembW_aug = din("bembW_aug", (K + 1, D))
    qkvW_aug_d = din("qkvW_aug", (L, D + 1, 3 * HHD), BF16)
    dWf0_aug_d = din("dWf0_aug", (D + 1, HHD))
    diffW_dup_d = din("diffW_dup", (L, 2 * D, HHD), BF16)
    diffb_d = din("diffb_cols", (L, HD, H))
    boutW_dup_d = din("boutW_dup", (L, HD, 8 * D), BF16)
    boutb_d = din("boutb2", (HD, L))
    oW_d = din("oW", (L, HHD, D), BF16)
    ob_d = din("ob_cols", (D, L))
    outW_aug_d = din("outW_aug", (D + 1, 1))
    ln1g_d = din("ln1g_t", (HD, D))
    ln1b_d = din("ln1b_t", (HD, D))
    ln2g_d = din("ln2g_t", (HD, D))
    ln2b_d = din("ln2b_t", (HD, D))
    strip_d = din("strip", (HD, 255), BF16)

    out_dram = nc.dram_tensor("out_loc", [R, 1], F32, kind="ExternalOutput").ap()

    with nc.allow_low_precision(reason="bf16 mish rational chain"), \
         tile.TileContext(nc) as tc, ExitStack() as ctx:
        cpool = ctx.enter_context(tc.tile_pool(name="const", bufs=1))
        ppool = ctx.enter_context(tc.tile_pool(name="persist", bufs=1))
        wpool = ctx.enter_context(tc.tile_pool(name="work", bufs=2))
        w2pool = ctx.enter_context(tc.tile_pool(name="work2", bufs=2))
        w64 = ctx.enter_context(tc.tile_pool(name="work64", bufs=2))
        statp = ctx.enter_context(tc.tile_pool(name="stat", bufs=4))
        ps_be = ctx.enter_context(tc.tile_pool(name="ps_be", bufs=2, space="PSUM"))
        ps_d = ctx.enter_context(tc.tile_pool(name="ps_d", bufs=1, space="PSUM"))
        ps_bn = ctx.enter_context(tc.tile_pool(name="ps_bn", bufs=1, space="PSUM"))
        ps_x = ctx.enter_context(tc.tile_pool(name="ps_x", bufs=1, space="PSUM"))
        dram = ctx.enter_context(tc.tile_pool(name="dram", bufs=1, space="DRAM"))
        pools = {"stat": statp, "work64": w64}

        dma = nc.sync.dma_start

        # ---------------- constants into SBUF ----------------
        def cload(name, shape, src_ap, dt=F32):
            t = cpool.tile(list(shape), dt, tag=name, name=name)
            dma(t[:], src_ap)
            return t

        # node features transposed (3 K-chunks: 128/128/1)
        nfT0 = cload("nfT0", [128, N], nfT_aug[0:128, :])
        nfT1 = cload("nfT1", [128, N], nfT_aug[128:256, :])
        nfT2 = cload("nfT2", [1, N], nfT_aug[256:257, :])
        nfl0 = cload("nfl0", [128, R], nfT_loc[0:128, :])
        nfl1 = cload("nfl1", [128, R], nfT_loc[128:256, :])
        nfl2 = cload("nfl2", [1, R], nfT_loc[256:257, :])
        embW0 = cload("embW0", [128, D], embW_aug[0:128, :])
        embW1 = cload("embW1", [128, D], embW_aug[128:256, :])
        embW2 = cload("embW2", [1, D], embW_aug[256:257, :])
        amds_sb = cload("amds_sb", [K + 1, N], amdsT_aug[:, :])
        amdl_sb = cload("amdl_sb", [K + 1, R], amdsT_loc[:, :])
        bembW = cload("bembW", [K + 1, D], bembW_aug[:, :])
        dWf0 = cload("dWf0", [D + 1, HHD], dWf0_aug_d[:, :])
        qkvW = [cload(f"qkvW{l}", [D + 1, 3 * HHD], qkvW_aug_d[l, :, :], BF16)
                for l in range(L)]
        diffW = [cload(f"diffW{l}", [2 * D, HHD], diffW_dup_d[l, :, :], BF16)
                 for l in range(1, L)]
        diffW = [None] + diffW
        diffb = [cload(f"diffb{l}", [HD, H], diffb_d[l, :, :])
                 for l in range(L)] if diffb_nonzero else None
        boutW = [cload(f"boutW{l}", [HD, 8 * D], boutW_dup_d[l, :, :], BF16)
                 for l in range(L - 1)]
        boutb = cload("boutb", [HD, L], boutb_d[:, :])
        oW_sb = []
        for l in range(L):
            t = cpool.tile([HD, H * D], BF16, tag=f"oW{l}", name=f"oW{l}")
            for h in range(H):
                dma(t[:, h * D:(h + 1) * D], oW_d[l, h * HD:(h + 1) * HD, :])
            oW_sb.append(t)
        ob_sb = cload("ob_sb", [D, L], ob_d[:, :])
        outW_sb = cload("outW_sb", [D + 1, 1], outW_aug_d[:, :])
        ln1g = cload("ln1g", [HD, D], ln1g_d[:, :])
        ln1b = cload("ln1b", [HD, D], ln1b_d[:, :])
        ln2g = cload("ln2g", [HD, D], ln2g_d[:, :])
        ln2b = cload("ln2b", [HD, D], ln2b_d[:, :])
        strip = cload("strip", [HD, 255], strip_d[:, :], BF16)

        ident = cpool.tile([128, 128], F32, tag="ident")
        make_identity(nc, ident[:])
        identb = cpool.tile([128, 128], BF16, tag="identb")
        make_identity(nc, identb[:])
        epsc = cpool.tile([128, 1], F32, tag="epsc")
        nc.gpsimd.memset(epsc[:], EPS)
        pools["eps"] = epsc
        onec = cpool.tile([128, 1], F32, tag="onec")
        nc.gpsimd.memset(onec[:], 1.0)

        # ---------------- persistent tiles ----------------
        biasA = ppool.tile([128, R * HHD // 2], BF16, tag="biasA")
        biasB = ppool.tile([128, R * HHD // 2], BF16, tag="biasB")
        b0L = ppool.tile([D, R], F32, tag="b0L")
        b0Tb = ppool.tile([D + 1, N], BF16, tag="b0Tb")
        bias0 = [ppool.tile([D + 1, N], BF16, tag=f"bias0_{par}",
                            name=f"bias0_{par}") for par in range(2)]
        xT = ppool.tile([D + 1, N], BF16, tag="xT")
        xlocT = ppool.tile([D + 1, R], BF16, tag="xlocT")
        x_loc = ppool.tile([R, D], F32, tag="x_loc")
        resid_loc = ppool.tile([R, D], F32, tag="resid_loc")
        pre_all = ppool.tile([128, 4 * D], F32, tag="pre_all")
        xfull = ppool.tile([128, 4 * D], F32, tag="xfull")
        kT = ppool.tile([HD, H * N], BF16, tag="kT")
        v_all = ppool.tile([128, H * HD * 4 // 4 * 4], BF16, tag="v_all")  # [128, 2048]
        ql = ppool.tile([HD, H * R], BF16, tag="ql")
        va = ppool.tile([HD, H * R], BF16, tag="va")
        diffs_s = [ppool.tile([128, N], F32, tag=f"diffs{p}", name=f"diffs{p}")
                   for p in range(2)]
        xfT = ppool.tile([D + 1, R], F32, tag="xfT")

        # collective bounce buffers
        gin = [dram.tile([R, D], F32, tag=f"gin{l}", name=f"gin{l}")
               for l in range(L - 1)]
        gout = [dram.tile([N, D], F32, tag=f"gout{l}", name=f"gout{l}")
                for l in range(L - 1)]

        # ---------------- head: h, b0, G ----------------
        # full pre-activation h rows -> pre_all ([128, 64] x 4 tiles)
        for m in range(4):
            ph = ps_x.tile([128, D], F32, tag="x")
            nc.tensor.matmul(ph[:], nfT0[:, m * 128:(m + 1) * 128], embW0[:],
                             start=True, stop=False)
            nc.tensor.matmul(ph[:], nfT1[:, m * 128:(m + 1) * 128], embW1[:],
                             start=False, stop=False)
            nc.tensor.matmul(ph[:], nfT2[:, m * 128:(m + 1) * 128], embW2[:],
                             start=False, stop=True)
            nc.vector.tensor_copy(out=pre_all[:, m * D:(m + 1) * D], in_=ph[:])
        # local pre-activation rows -> resid_loc
        pl = ps_x.tile([R, D], F32, tag="x")
        nc.tensor.matmul(pl[:], nfl0[:], embW0[:], start=True, stop=False)
        nc.tensor.matmul(pl[:], nfl1[:], embW1[:], start=False, stop=False)
        nc.tensor.matmul(pl[:], nfl2[:], embW2[:], start=False, stop=True)
        nc.vector.tensor_copy(resid_loc[:], pl[:])
        # b0 transposed (bf16, augmented ones row) and local columns
        pb = ps_x.tile([D, N], F32, tag="x")
        nc.tensor.matmul(pb[:], bembW[:], amds_sb[:], start=True, stop=True)
        nc.vector.tensor_copy(out=b0Tb[0:D, :], in_=pb[:])
        nc.gpsimd.memset(b0Tb[D:D + 1, :], 1.0)
        pbl = ps_x.tile([D, R], F32, tag="x")
        nc.tensor.matmul(pbl[:], bembW[:], amdl_sb[:], start=True, stop=True)
        nc.vector.tensor_copy(b0L[:], pbl[:])
        # bf16 copy of the augmented diff_W[0] for the l=0 per-row matmuls
        dWf0b = cpool.tile([D + 1, HHD], BF16, tag="dWf0b", name="dWf0b")
        nc.vector.tensor_copy(out=dWf0b[:], in_=dWf0[:])
        for par in range(2):
            nc.gpsimd.memset(bias0[par][D:D + 1, :], 1.0)

        def _early_out():
            osb_e = w64.tile([R, 1], F32, tag="osb", name="osb_e")
            nc.vector.tensor_copy(osb_e[:], resid_loc[:, 0:1])
            nc.sync.dma_start(out_dram[:, :], osb_e[:])

        if trunc == 1:
            _early_out()
        n_layers = L if trunc == 0 else min(L, trunc - 1)

        # ---------------- layers ----------------
        for l in range(n_layers):
            bias_cur = biasA if l in (1, 3) else biasB
            bias_nxt = biasA if l == 0 else biasB if l == 1 else biasA

            # ---- (a) i-loop: bias chain ----
            # mish(x) = x*(1 - 2r), r = 1/(u^2+2u+2), u = e^x.  r is computed
            # as exp(-ln(w+2)) on the scalar LUT (exp+ln live in one table),
            # the final multiply as one AFFINE_MUL_REDUCE custom-DVE op.
            # Processed in half tiles [128, 2N] (head pairs) so the be-psum
            # can double-buffer (2 bufs x 2 banks).
            psum_bn = None
            psum_diff = [ps_d.tile([128, N], F32, tag=f"d{q}", name=f"pd{l}_{q}")
                         for q in range(2)]
            for i in range(R):
                half = (i % 2) * D
                for s in range(2):
                    psum_be = ps_be.tile([128, 2 * N], F32, tag="be")
                    if l == 0:
                        bias_t = bias0[i % 2]
                        if s == 0:
                            nc.vector.tensor_scalar(
                                bias_t[0:D, :], b0Tb[0:D, :],
                                b0L[:, i:i + 1], None, ALU.subtract)
                        for mm in range(2):
                            m = 2 * s + mm
                            nc.tensor.matmul(
                                psum_be[:, mm * N:(mm + 1) * N],
                                dWf0b[:, m * 128:(m + 1) * 128],
                                bias_t[:, :], start=True, stop=True)
                    else:
                        for mm in range(2):
                            m = 2 * s + mm
                            nc.tensor.matmul(
                                psum_be[:, mm * N:(mm + 1) * N],
                                diffW[l][half:half + D, m * 128:(m + 1) * 128],
                                bias_cur[half:half + D,
                                         (i // 2) * HHD:(i // 2) * HHD + HHD],
                                start=True, stop=True)
                    u_t = wpool.tile([128, 2 * N], BF16, tag="u",
                                     name=f"u{l}_{i}_{s}")
                    if l > 0 and diffb_nonzero:
                        xb = wpool.tile([128, 2 * N], BF16, tag="xb",
                                        name=f"xb{l}_{i}_{s}")
                        for mm in range(2):
                            m = 2 * s + mm
                            sl = slice(mm * N, (mm + 1) * N)
                            nc.scalar.activation(xb[:, sl], psum_be[:, sl],
                                                 AF.Identity,
                                                 bias=diffb[l][:, m:m + 1])
                        nc.scalar.activation(u_t[:], xb[:], AF.Exp)
                        x_src = xb
                    else:
                        nc.scalar.activation(u_t[:], psum_be[:], AF.Exp)
                        x_src = psum_be
                    # p = (u+1)^2 ; d = p+1 = u^2+2u+2 ; r ~= 1/d ;
                    # mish = (r*(-2)+1) * x   (one custom-DVE op)
                    p_t = wpool.tile([128, 2 * N], F32, tag="p",
                                     name=f"p{l}_{i}_{s}")
                    nc.scalar.activation(p_t[:], u_t[:], AF.Square,
                                         bias=onec[:])
                    d_t = wpool.tile([128, 2 * N], F32, tag="d",
                                     name=f"d{l}_{i}_{s}")
                    nc.vector.tensor_scalar(d_t[:], p_t[:], 1.0, None, ALU.add)
                    r_t = wpool.tile([128, 2 * N], F32, tag="r",
                                     name=f"r{l}_{i}_{s}")
                    nc.vector.reciprocal_approx_fast(out=r_t[:], in_=d_t[:])
                    mish_t = wpool.tile([128, 2 * N], BF16, tag="mish",
                                        name=f"mish{l}_{i}_{s}")
                    nc.vector._custom_dve(
                        AFFINE_MUL_REDUCE, out=mish_t[:], in0=r_t[:],
                        in1=x_src[:], s0=-2.0, s1=1.0)
                    sq_t = wpool.tile([128, 2 * N], BF16, tag="sq",
                                      name=f"sq{l}_{i}_{s}")
                    nc.scalar.activation(sq_t[:], mish_t[:], AF.Square)
                    # diffs accumulation (one-hot column matmuls): half s
                    # feeds head pair p == s
                    for hh in range(2):
                        col = hh * D + i
                        nc.tensor.matmul(
                            psum_diff[s][:],
                            strip[:, 127 - col:255 - col],
                            sq_t[:, hh * N:(hh + 1) * N],
                            start=(i == 0 and hh == 0),
                            stop=(i == R - 1 and hh == 1),
                            skip_group_check=True)
                    # next-layer bias (skip on last layer)
                    if l < L - 1:
                        if s == 0 and i % 2 == 0:
                            psum_bn = ps_bn.tile([128, HHD], F32, tag="bn",
                                                 name=f"bn{l}_{i}")
                        for mm in range(2):
                            m = 2 * s + mm
                            nc.tensor.matmul(
                                psum_bn[half:half + D, :],
                                boutW[l][:, m * 128 + half:m * 128 + half + D],
                                mish_t[:, mm * N:(mm + 1) * N],
                                start=(m == 0), stop=(m == 3),
                                tile_position=(0, half))
                if l < L - 1 and i % 2 == 1:
                    # mish on the accumulated [128, HHD] bias tile
                    bsl = slice((i // 2) * HHD, (i // 2) * HHD + HHD)
                    u2 = w2pool.tile([128, HHD], BF16, tag="u2",
                                     name=f"u2_{l}_{i}")
                    nc.scalar.activation(u2[:], psum_bn[:], AF.Exp,
                                         bias=boutb[:, l:l + 1])
                    p2 = w2pool.tile([128, HHD], F32, tag="p2",
                                     name=f"p2_{l}_{i}")
                    nc.scalar.activation(p2[:], u2[:], AF.Square,
                                         bias=onec[:])
                    d2 = w2pool.tile([128, HHD], F32, tag="d2",
                                     name=f"d2_{l}_{i}")
                    nc.vector.tensor_scalar(d2[:], p2[:], 1.0, None, ALU.add)
                    r2 = w2pool.tile([128, HHD], F32, tag="r2",
                                     name=f"r2_{l}_{i}")
                    nc.vector.reciprocal_approx_fast(out=r2[:], in_=d2[:])
                    tm2 = w2pool.tile([128, HHD], BF16, tag="tm2",
                                      name=f"tm2_{l}_{i}")
                    nc.vector.tensor_scalar(tm2[:], r2[:], -2.0, 1.0,
                                            ALU.mult, ALU.add)
                    nc.vector._custom_dve(
                        AFFINE_MUL_REDUCE, out=bias_nxt[:, bsl],
                        in0=psum_bn[:], in1=tm2[:], s0=1.0,
                        s1=boutb[:, l:l + 1])

            # ---- (b) sqrt window: diffs sqrt + LN -> x_l ----
            for p in range(2):
                nc.scalar.activation(diffs_s[p][:], psum_diff[p][:], AF.Sqrt)
            if l == n_layers - 1 and trunc != 0 and os.environ.get("KHALF") == "1":
                break
            if l > 0:
                for m in range(4):
                    dma(pre_all[:, m * D:(m + 1) * D],
                        gout[l - 1][m * 128:(m + 1) * 128, :])
            g_t, b_t = (ln1g, ln1b) if l == 0 else (ln2g, ln2b)
            for m in range(4):
                _ln_tiles(nc, tc, pools, pre_all[:, m * D:(m + 1) * D], 128,
                          g_t[:], b_t[:], xfull[:, m * D:(m + 1) * D])
            _ln_tiles(nc, tc, pools, resid_loc[:], R,
                      g_t[0:R, :], b_t[0:R, :], x_loc[:])
            if l == n_layers - 1 and trunc != 0 and \
                    int(os.environ.get("KPHASE", "9")) <= 0:
                break
            # transposes -> xT (augmented), xlocT (augmented)
            for m in range(4):
                pt = ps_x.tile([D, 128], F32, tag="x")
                nc.tensor.transpose(pt[:], xfull[:, m * D:(m + 1) * D], ident[:])
                nc.vector.tensor_copy(out=xT[0:D, m * 128:(m + 1) * 128],
                                      in_=pt[:])
            nc.gpsimd.memset(xT[D:D + 1, :], 1.0)
            ptl = ps_x.tile([D, R], F32, tag="x")
            nc.tensor.transpose(ptl[:], x_loc[:], ident[0:R, 0:R])
            nc.vector.tensor_copy(out=xlocT[0:D, :], in_=ptl[:])
            nc.gpsimd.memset(xlocT[D:D + 1, :], 1.0)
            if l == n_layers - 1 and trunc != 0 and \
                    int(os.environ.get("KPHASE", "9")) <= 1:
                break

            # ---- (c) qkv ----
            for h in range(H):
                base = h * 3 * HD
                # k^T for head h
                pk = ps_x.tile([HD, N], F32, tag="x")
                nc.tensor.matmul(pk[:], qkvW[l][:, base + HD:base + 2 * HD],
                                 xT[:], start=True, stop=True)
                nc.vector.tensor_copy(out=kT[:, h * N:(h + 1) * N], in_=pk[:])
                # q^T local rows
                pq = ps_x.tile([HD, R], F32, tag="x")
                nc.tensor.matmul(pq[:], qkvW[l][:, base:base + HD],
                                 xlocT[:], start=True, stop=True)
                nc.vector.tensor_copy(out=ql[:, h * R:(h + 1) * R], in_=pq[:])
                # v (untransposed) per token chunk
                for tc_ in range(4):
                    pv = ps_x.tile([128, HD], F32, tag="x")
                    nc.tensor.matmul(pv[:], xT[:, tc_ * 128:(tc_ + 1) * 128],
                                     qkvW[l][:, base + 2 * HD:base + 3 * HD],
                                     start=True, stop=True)
                    nc.vector.tensor_copy(
                        out=v_all[:, (h * 4 + tc_) * HD:(h * 4 + tc_ + 1) * HD],
                        in_=pv[:])

            if l == n_layers - 1 and trunc != 0 and \
                    int(os.environ.get("KPHASE", "9")) <= 2:
                break
            # ---- (d) attention per head ----
            for h in range(H):
                p, hh = h // 2, h % 2
                plg = ps_x.tile([R, N], F32, tag="x")
                nc.tensor.matmul(plg[:], ql[:, h * R:(h + 1) * R],
                                 kT[:, h * N:(h + 1) * N], start=True, stop=True)
                pre_sb = wpool.tile([R, N], BF16, tag="pre_sb")
                nc.vector.scalar_tensor_tensor(
                    out=pre_sb[:], in0=plg[:], scalar=SCALE,
                    in1=diffs_s[p][hh * R:(hh + 1) * R, :],
                    op0=ALU.mult, op1=ALU.add)
                nmax = statp.tile([R, 1], F32, tag="nmax")
                nc.vector.reduce_max(nmax[:], pre_sb[:], axis=AX.X, negate=True)
                esb = wpool.tile([R, N], BF16, tag="esb")
                sumexp = statp.tile([R, 1], F32, tag="sumexp")
                nc.scalar.activation(esb[:], pre_sb[:], AF.Exp,
                                     bias=nmax[:], accum_out=sumexp[:])
                rsum = statp.tile([R, 1], F32, tag="rsum")
                nc.vector.reciprocal(rsum[:], sumexp[:])
                att = wpool.tile([R, N], BF16, tag="att")
                nc.vector.tensor_scalar(att[:], esb[:], rsum[:], None, ALU.mult)
                attT = wpool.tile([128, 4 * R], BF16, tag="attT")
                for tc_ in range(4):
                    pat = ps_x.tile([128, R], BF16, tag="x")
                    nc.tensor.transpose(pat[:], att[:, tc_ * 128:(tc_ + 1) * 128],
                                        identb[0:R, 0:R])
                    nc.vector.tensor_copy(out=attT[:, tc_ * R:(tc_ + 1) * R],
                                          in_=pat[:])
                pvl = ps_x.tile([HD, R], F32, tag="x")
                for tc_ in range(4):
                    nc.tensor.matmul(
                        pvl[:],
                        v_all[:, (h * 4 + tc_) * HD:(h * 4 + tc_ + 1) * HD],
                        attT[:, tc_ * R:(tc_ + 1) * R],
                        start=(tc_ == 0), stop=(tc_ == 3))
                nc.vector.tensor_copy(out=va[:, h * R:(h + 1) * R], in_=pvl[:])

            if l == n_layers - 1 and trunc != 0 and \
                    int(os.environ.get("KPHASE", "9")) <= 3:
                break
            # ---- (e) output projection for local rows ----
            ptx = ps_x.tile([D, R], F32, tag="x")
            for h in range(H):
                nc.tensor.matmul(ptx[:], oW_sb[l][:, h * D:(h + 1) * D],
                                 va[:, h * R:(h + 1) * R],
                                 start=(h == 0), stop=(h == 3))
            tempxT = w64.tile([D, R], F32, tag="tempxT")
            nc.scalar.activation(tempxT[:], ptx[:], AF.Identity,
                                 bias=ob_sb[:, l:l + 1])
            # residual: resid_loc = x_loc + temp_x (untransposed)
            ptu = ps_x.tile([R, D], F32, tag="x")
            nc.tensor.transpose(ptu[:], tempxT[:], ident[0:D, 0:D])
            nc.vector.tensor_tensor(resid_loc[:], ptu[:], x_loc[:], ALU.add)

            # ---- (f) gather residual rows (layers 0-2) ----
            if l == n_layers - 1 and trunc != 0 and \
                    int(os.environ.get("KPHASE", "9")) <= 4:
                break
            if l < L - 1:
                nc.sync.dma_start(gin[l][:], resid_loc[:])
                nc.gpsimd.collective_compute(
                    "AllGather", ALU.bypass,
                    replica_groups=[list(range(NCORES))],
                    ins=[gin[l].opt()], outs=[gout[l].opt()])

        # ---------------- final: LN + out head on local rows ----------------
        if trunc > 1:
            _early_out()
        if trunc == 0:
            x4 = w64.tile([R, D], F32, tag="x4")
            _ln_tiles(nc, tc, pools, resid_loc[:], R, ln2g[0:R, :],
                      ln2b[0:R, :], x4[:])
            pxf = ps_x.tile([D, R], F32, tag="x")
            nc.tensor.transpose(pxf[:], x4[:], ident[0:R, 0:R])
            nc.vector.tensor_copy(out=xfT[0:D, :], in_=pxf[:])
            nc.gpsimd.memset(xfT[D:D + 1, :], 1.0)
            pout = ps_x.tile([R, 1], F32, tag="x")
            nc.tensor.matmul(pout[:], xfT[:], outW_sb[:], start=True, stop=True)
            osb = w64.tile([R, 1], F32, tag="osb")
            nc.vector.tensor_copy(osb[:], pout[:])
            nc.sync.dma_start(out_dram[:, :], osb[:])

    nc.compile()
    return nc


def _prep_inputs(inputs):
    f32 = np.float32

    def f(x):
        return np.ascontiguousarray(np.asarray(x), dtype=f32)

    nf = f(inputs["node_features"])
    amds = f(inputs["amds"])
    emb_W, emb_b = f(inputs["emb_W"]), f(inputs["emb_b"])
    bemb_W, bemb_b = f(inputs["bias_emb_W"]), f(inputs["bias_emb_b"])
    qkv_W, qkv_b = f(inputs["qkv_W"]), f(inputs["qkv_b"])
    diff_W, diff_b = f(inputs["diff_W"]), f(inputs["diff_b"])
    o_W, o_b = f(inputs["o_W"]), f(inputs["o_b"])
    bout_W, bout_b = f(inputs["bout_W"]), f(inputs["bout_b"])
    out_W, out_b = f(inputs["out_W"]), f(inputs["out_b"])
    ln1_g, ln1_b = f(inputs["ln1_g"]), f(inputs["ln1_b"])
    ln2_g, ln2_b = f(inputs["ln2_g"]), f(inputs["ln2_b"])

    ones_n = np.ones((1, N), f32)
    ones_r = np.ones((1, R), f32)
    com = {}
    com["nfT_aug"] = np.ascontiguousarray(
        np.concatenate([nf.T, ones_n], 0))
    com["amdsT_aug"] = np.ascontiguousarray(
        np.concatenate([amds.T, ones_n], 0))
    com["embW_aug"] = np.concatenate([emb_W, emb_b[None, :]], 0)
    com["bembW_aug"] = np.concatenate([bemb_W, bemb_b[None, :]], 0)
    com["qkvW_aug"] = np.ascontiguousarray(
        np.concatenate([qkv_W, qkv_b[:, None, :]], 1)).astype(NP_BF16)
    com["dWf0_aug"] = np.concatenate([diff_W[0], diff_b[0][None, :]], 0)
    com["diffW_dup"] = np.ascontiguousarray(
        np.concatenate([diff_W, diff_W], 1)).astype(NP_BF16)
    com["diffb_cols"] = np.ascontiguousarray(
        diff_b.reshape(L, H, HD).transpose(0, 2, 1))
    bwd = np.zeros((L, HD, 8 * D), f32)
    for l in range(L):
        for h in range(H):
            chunk = bout_W[l, h * HD:(h + 1) * HD, :]  # [128, 64]
            bwd[l, :, h * 2 * D:h * 2 * D + D] = chunk
            bwd[l, :, h * 2 * D + D:h * 2 * D + 2 * D] = chunk
    com["boutW_dup"] = bwd.astype(NP_BF16)
    com["boutb2"] = np.ascontiguousarray(
        np.tile(bout_b, (1, 2)).T)  # [128, L]
    com["oW"] = o_W.astype(NP_BF16)
    com["ob_cols"] = np.ascontiguousarray(o_b.T)
    com["outW_aug"] = np.concatenate([out_W, out_b[None, :]], 0)
    com["ln1g_t"] = np.tile(ln1_g[None, :], (HD, 1))
    com["ln1b_t"] = np.tile(ln1_b[None, :], (HD, 1))
    com["ln2g_t"] = np.tile(ln2_g[None, :], (HD, 1))
    com["ln2b_t"] = np.tile(ln2_b[None, :], (HD, 1))
    strip = np.zeros((HD, 255), f32)
    strip[:, 127] = 1.0
    com["strip"] = strip.astype(NP_BF16)

    in_maps = []
    for c in range(NCORES):
        m = dict(com)
        m["nfT_loc"] = np.ascontiguousarray(
            np.concatenate([nf.T[:, c * R:(c + 1) * R], ones_r], 0))
        m["amdsT_loc"] = np.ascontiguousarray(
            np.concatenate([amds.T[:, c * R:(c + 1) * R], ones_r], 0))
        in_maps.append(m)
    diffb_nonzero = bool(np.any(diff_b != 0.0))
    return in_maps, diffb_nonzero


_LAST_RESULTS = None


def kernel(**inputs) -> np.ndarray:
    global _LAST_RESULTS
    in_maps, diffb_nonzero = _prep_inputs(inputs)
    trunc = int(os.environ.get("KTRUNC", "0"))
    nc = _build(diffb_nonzero, trunc)
    trace = bool(int(os.environ.get("KERNEL_TRACE", "0")))
    try:
        res = bass_utils.run_bass_kernel_spmd(
            nc, in_maps, core_ids=list(range(NCORES)), trace=trace)
    except ModuleNotFoundError:
        res = bass_utils.run_bass_kernel_spmd(
            nc, in_maps, core_ids=list(range(NCORES)), trace=False)
    _LAST_RESULTS = res
    out = np.concatenate(
        [res.results[c]["out_loc"] for c in range(NCORES)], axis=0)
    return out.astype(np.float32)


if __name__ == "__main__":
    rng = np.random.default_rng(0)
    dummy = {
        "node_features": rng.standard_normal((N, FB), dtype=np.float32),
        "amds": rng.random((N, K), dtype=np.float32),
        "emb_W": rng.standard_normal((FB, D), dtype=np.float32) / 16,
        "emb_b": np.zeros((D,), np.float32),
        "bias_emb_W": rng.standard_normal((K, D), dtype=np.float32) / 10,
        "bias_emb_b": np.zeros((D,), np.float32),
        "ln1_g": np.ones((D,), np.float32),
        "ln1_b": np.zeros((D,), np.float32),
        "ln2_g": np.ones((D,), np.float32),
        "ln2_b": np.zeros((D,), np.float32),
        "qkv_W": rng.standard_normal((L, D, 3 * HHD), dtype=np.float32) / 8,
        "qkv_b": np.zeros((L, 3 * HHD), np.float32),
        "diff_W": rng.standard_normal((L, D, HHD), dtype=np.float32) / 8,
        "diff_b": np.zeros((L, HHD), np.float32),
        "o_W": rng.standard_normal((L, HHD, D), dtype=np.float32) / 22,
        "o_b": np.zeros((L, D), np.float32),
        "bout_W": rng.standard_normal((L, HHD, D), dtype=np.float32) / 22,
        "bout_b": np.zeros((L, D), np.float32),
        "out_W": rng.standard_normal((D, 1), dtype=np.float32) / 8,
        "out_b": np.zeros((1,), np.float32),
    }
    out = kernel(**dummy)
    print("kernel output shape:", out.shape, "first:", out[:4, 0])



# revision 26
# speedup vs baseline: 1.1968x; 1.0193x over previous
"""Trainium2 Bass kernel for nn_CrAKN (dense transformer with pairwise bias chain).

Sharding: rows of the N=512 crystal dimension are split across 8 cores
(64 rows each). Each core computes its [64, N, 512] bias-chain slice and its
64 attention rows; per layer the updated residual rows are AllGathered so
every core can form the full k/v for the next layer.

Self-contained: hardcodes all shapes; builds one SPMD Bass program and runs
it via run_bass_kernel_spmd on cores 0-7.
"""

import os
import sys
import functools
from contextlib import ExitStack

import numpy as np

sys.path.insert(0, "/opt/trn_rl_repo")

import concourse.bass as bass  # noqa: E402
import concourse.bacc as bacc  # noqa: E402
import concourse.tile as tile  # noqa: E402
import concourse.mybir as mybir  # noqa: E402
import concourse.bass_utils as bass_utils  # noqa: E402
from concourse.masks import make_identity  # noqa: E402
from concourse.dve_ops import AFFINE_MUL_REDUCE  # noqa: E402

F32 = mybir.dt.float32
BF16 = mybir.dt.bfloat16
NP_BF16 = mybir.dt.np(BF16)

AF = mybir.ActivationFunctionType
ALU = mybir.AluOpType
AX = mybir.AxisListType

N, FB, D, H, HD, L, K = 512, 256, 64, 128, 4, 4, 100
H, HD = 4, 128
HHD = H * HD  # 512
NCORES = 8
R = N // NCORES  # 64 rows per core
EPS = 1e-5
SCALE = 1.0 / float(np.sqrt(HD))


def _ln_batch(nc, pools, insts, tag):
    """Batched LayerNorm: insts = list of (in_ap, parts, g_ap, b_ap, out_ap).
    All variances collect into one [128, K] tile so a single scalar Sqrt
    serves every instance (avoids act-table thrashing)."""
    stat = pools["stat"]
    work = pools["work64"]
    K = len(insts)
    varb = stat.tile([128, K], F32, tag="ln_varb", name=f"varb_{tag}")
    cens = []
    for k, (in_ap, parts, g_ap, b_ap, out_ap) in enumerate(insts):
        ssum = stat.tile([parts, 1], F32, tag="ln_sum")
        nc.vector.reduce_sum(ssum[:], in_ap, axis=AX.X)
        mu = stat.tile([parts, 1], F32, tag="ln_mu")
        nc.vector.tensor_scalar(mu[:], ssum[:], 1.0 / D, None, ALU.mult)
        cen = work.tile([parts, D], F32, tag=f"ln_cen{k}",
                        name=f"cen_{tag}_{k}")
        nc.vector.tensor_scalar(cen[:], in_ap, mu[:], None, ALU.subtract)
        vscr = work.tile([parts, D], F32, tag="ln_vscr")
        nc.vector.tensor_tensor(vscr[:], cen[:], cen[:], ALU.mult)
        nc.vector.reduce_sum(varb[0:parts, k:k + 1], vscr[:], axis=AX.X)
        cens.append(cen)
    sd = stat.tile([128, K], F32, tag="ln_sd", name=f"sd_{tag}")
    nc.scalar.activation(sd[:], varb[:], AF.Sqrt, scale=1.0 / D,
                         bias=pools["eps"])
    rsb = stat.tile([128, K], F32, tag="ln_rs", name=f"rs_{tag}")
    nc.vector.reciprocal(rsb[:], sd[:])
    for k, (in_ap, parts, g_ap, b_ap, out_ap) in enumerate(insts):
        xg = work.tile([parts, D], F32, tag="ln_xg")
        nc.vector.scalar_tensor_tensor(
            out=xg[:], in0=cens[k][:], scalar=rsb[0:parts, k:k + 1],
            in1=g_ap, op0=ALU.mult, op1=ALU.mult)
        nc.vector.tensor_tensor(out_ap, xg[:], b_ap, ALU.add)


@functools.lru_cache(maxsize=4)
def _build(diffb_nonzero: bool, trunc: int = 0):
    nc = bacc.Bacc("TRN2", target_bir_lowering=False, debug=False,
                   enable_asserts=False, num_devices=NCORES)

    def din(name, shape, dt=F32):
        return nc.dram_tensor(name, list(shape), dt, kind="ExternalInput").ap()

    nfT_aug = din("nfT_aug", (FB + 1, N))
    nfT_loc = din("nfT_loc", (FB + 1, R))
    amdsT_aug = din("amdsT_aug", (K + 1, N))
    amdsT_loc = din("amdsT_loc", (K + 1, R))
    embW_aug = din("embW_aug", (FB + 1, D))
    bembW_aug = din("bembW_aug", (K + 1, D))
    qkvW_aug_d = din("qkvW_aug", (L, D + 1, 3 * HHD), BF16)
    dWf0_aug_d = din("dWf0_aug", (D + 1, HHD))
    diffW_dup_d = din("diffW_dup", (L, 2 * D, HHD), BF16)
    diffb_d = din("diffb_cols", (L, HD, H))
    boutW_dup_d = din("boutW_dup", (L, HD, 8 * D), BF16)
    boutb_d = din("boutb2", (HD, L))
    oW_d = din("oW", (L, HHD, D), BF16)
    ob_d = din("ob_cols", (D, L))
    outW_aug_d = din("outW_aug", (D + 1, 1))
    ln1g_d = din("ln1g_t", (HD, D))
    ln1b_d = din("ln1b_t", (HD, D))
    ln2g_d = din("ln2g_t", (HD, D))
    ln2b_d = din("ln2b_t", (HD, D))
    strip_d = din("strip", (HD, 255), BF16)

    out_dram = nc.dram_tensor("out_loc", [R, 1], F32, kind="ExternalOutput").ap()

    with nc.allow_low_precision(reason="bf16 mish rational chain"), \
         tile.TileContext(nc) as tc, ExitStack() as ctx:
        cpool = ctx.enter_context(tc.tile_pool(name="const", bufs=1))
        ppool = ctx.enter_context(tc.tile_pool(name="persist", bufs=1))
        wpool = ctx.enter_context(tc.tile_pool(name="work", bufs=2))
        w2pool = ctx.enter_context(tc.tile_pool(name="work2", bufs=2))
        w64 = ctx.enter_context(tc.tile_pool(name="work64", bufs=2))
        statp = ctx.enter_context(tc.tile_pool(name="stat", bufs=4))
        ps_be = ctx.enter_context(tc.tile_pool(name="ps_be", bufs=2, space="PSUM"))
        ps_d = ctx.enter_context(tc.tile_pool(name="ps_d", bufs=1, space="PSUM"))
        ps_bn = ctx.enter_context(tc.tile_pool(name="ps_bn", bufs=1, space="PSUM"))
        ps_x = ctx.enter_context(tc.tile_pool(name="ps_x", bufs=1, space="PSUM"))
        dram = ctx.enter_context(tc.tile_pool(name="dram", bufs=1, space="DRAM"))
        pools = {"stat": statp, "work64": w64}

        dma = nc.sync.dma_start

        # ---------------- constants into SBUF ----------------
        def cload(name, shape, src_ap, dt=F32):
            t = cpool.tile(list(shape), dt, tag=name, name=name)
            dma(t[:], src_ap)
            return t

        # node features transposed (3 K-chunks: 128/128/1)
        nfT0 = cload("nfT0", [128, N], nfT_aug[0:128, :])
        nfT1 = cload("nfT1", [128, N], nfT_aug[128:256, :])
        nfT2 = cload("nfT2", [1, N], nfT_aug[256:257, :])
        nfl0 = cload("nfl0", [128, R], nfT_loc[0:128, :])
        nfl1 = cload("nfl1", [128, R], nfT_loc[128:256, :])
        nfl2 = cload("nfl2", [1, R], nfT_loc[256:257, :])
        embW0 = cload("embW0", [128, D], embW_aug[0:128, :])
        embW1 = cload("embW1", [128, D], embW_aug[128:256, :])
        embW2 = cload("embW2", [1, D], embW_aug[256:257, :])
        amds_sb = cload("amds_sb", [K + 1, N], amdsT_aug[:, :])
        amdl_sb = cload("amdl_sb", [K + 1, R], amdsT_loc[:, :])
        bembW = cload("bembW", [K + 1, D], bembW_aug[:, :])
        dWf0 = cload("dWf0", [D + 1, HHD], dWf0_aug_d[:, :])
        qkvW = [cload(f"qkvW{l}", [D + 1, 3 * HHD], qkvW_aug_d[l, :, :], BF16)
                for l in range(L)]
        diffW = [cload(f"diffW{l}", [2 * D, HHD], diffW_dup_d[l, :, :], BF16)
                 for l in range(1, L)]
        diffW = [None] + diffW
        diffb = [cload(f"diffb{l}", [HD, H], diffb_d[l, :, :])
                 for l in range(L)] if diffb_nonzero else None
        boutW = [cload(f"boutW{l}", [HD, 8 * D], boutW_dup_d[l, :, :], BF16)
                 for l in range(L - 1)]
        boutb = cload("boutb", [HD, L], boutb_d[:, :])
        oW_sb = []
        for l in range(L):
            t = cpool.tile([HD, H * D], BF16, tag=f"oW{l}", name=f"oW{l}")
            for h in range(H):
                dma(t[:, h * D:(h + 1) * D], oW_d[l, h * HD:(h + 1) * HD, :])
            oW_sb.append(t)
        ob_sb = cload("ob_sb", [D, L], ob_d[:, :])
        outW_sb = cload("outW_sb", [D + 1, 1], outW_aug_d[:, :])
        ln1g = cload("ln1g", [HD, D], ln1g_d[:, :])
        ln1b = cload("ln1b", [HD, D], ln1b_d[:, :])
        ln2g = cload("ln2g", [HD, D], ln2g_d[:, :])
        ln2b = cload("ln2b", [HD, D], ln2b_d[:, :])
        strip = cload("strip", [HD, 255], strip_d[:, :], BF16)

        ident = cpool.tile([128, 128], F32, tag="ident")
        make_identity(nc, ident[:])
        identb = cpool.tile([128, 128], BF16, tag="identb")
        make_identity(nc, identb[:])
        epsc = cpool.tile([128, 1], F32, tag="epsc")
        nc.gpsimd.memset(epsc[:], EPS)
        pools["eps"] = epsc
        onec = cpool.tile([128, 1], F32, tag="onec")
        nc.gpsimd.memset(onec[:], 1.0)

        # ---------------- persistent tiles ----------------
        biasA = ppool.tile([128, R * HHD // 2], BF16, tag="biasA")
        biasB = ppool.tile([128, R * HHD // 2], BF16, tag="biasB")
        b0L = ppool.tile([D, R], F32, tag="b0L")
        b0Tb = ppool.tile([D + 1, N], BF16, tag="b0Tb")
        bias0 = [ppool.tile([D + 1, N], BF16, tag=f"bias0_{par}",
                            name=f"bias0_{par}") for par in range(2)]
        xT = ppool.tile([D + 1, N], BF16, tag="xT")
        xlocT = ppool.tile([D + 1, R], BF16, tag="xlocT")
        x_loc = ppool.tile([R, D], F32, tag="x_loc")
        resid_loc = ppool.tile([R, D], F32, tag="resid_loc")
        pre_all = ppool.tile([128, 4 * D], F32, tag="pre_all")
        xfull = ppool.tile([128, 4 * D], F32, tag="xfull")
        kT = ppool.tile([HD, H * N], BF16, tag="kT")
        v_all = ppool.tile([128, H * HD * 4 // 4 * 4], BF16, tag="v_all")  # [128, 2048]
        ql = ppool.tile([HD, H * R], BF16, tag="ql")
        va = ppool.tile([HD, H * R], BF16, tag="va")
        diffs_s = [ppool.tile([128, N], F32, tag=f"diffs{p}", name=f"diffs{p}")
                   for p in range(2)]
        xfT = ppool.tile([D + 1, R], F32, tag="xfT")

        # collective bounce buffers
        gin = [dram.tile([R, D], F32, tag=f"gin{l}", name=f"gin{l}")
               for l in range(L - 1)]
        gout = [dram.tile([N, D], F32, tag=f"gout{l}", name=f"gout{l}")
                for l in range(L - 1)]

        # ---------------- head: h, b0, G ----------------
        # full pre-activation h rows -> pre_all ([128, 64] x 4 tiles)
        for m in range(4):
            ph = ps_x.tile([128, D], F32, tag="x")
            nc.tensor.matmul(ph[:], nfT0[:, m * 128:(m + 1) * 128], embW0[:],
                             start=True, stop=False)
            nc.tensor.matmul(ph[:], nfT1[:, m * 128:(m + 1) * 128], embW1[:],
                             start=False, stop=False)
            nc.tensor.matmul(ph[:], nfT2[:, m * 128:(m + 1) * 128], embW2[:],
                             start=False, stop=True)
            nc.vector.tensor_copy(out=pre_all[:, m * D:(m + 1) * D], in_=ph[:])
        # local pre-activation rows -> resid_loc
        pl = ps_x.tile([R, D], F32, tag="x")
        nc.tensor.matmul(pl[:], nfl0[:], embW0[:], start=True, stop=False)
        nc.tensor.matmul(pl[:], nfl1[:], embW1[:], start=False, stop=False)
        nc.tensor.matmul(pl[:], nfl2[:], embW2[:], start=False, stop=True)
        nc.vector.tensor_copy(resid_loc[:], pl[:])
        # b0 transposed (bf16, augmented ones row) and local columns
        pb = ps_x.tile([D, N], F32, tag="x")
        nc.tensor.matmul(pb[:], bembW[:], amds_sb[:], start=True, stop=True)
        nc.vector.tensor_copy(out=b0Tb[0:D, :], in_=pb[:])
        nc.gpsimd.memset(b0Tb[D:D + 1, :], 1.0)
        pbl = ps_x.tile([D, R], F32, tag="x")
        nc.tensor.matmul(pbl[:], bembW[:], amdl_sb[:], start=True, stop=True)
        nc.vector.tensor_copy(b0L[:], pbl[:])
        # bf16 copy of the augmented diff_W[0] for the l=0 per-row matmuls
        dWf0b = cpool.tile([D + 1, HHD], BF16, tag="dWf0b", name="dWf0b")
        nc.vector.tensor_copy(out=dWf0b[:], in_=dWf0[:])
        for par in range(2):
            nc.gpsimd.memset(bias0[par][D:D + 1, :], 1.0)

        def _early_out():
            osb_e = w64.tile([R, 1], F32, tag="osb", name="osb_e")
            nc.vector.tensor_copy(osb_e[:], resid_loc[:, 0:1])
            nc.sync.dma_start(out_dram[:, :], osb_e[:])

        if trunc == 1:
            _early_out()
        n_layers = L if trunc == 0 else min(L, trunc - 1)

        # ---------------- layers ----------------
        for l in range(n_layers):
            bias_cur = biasA if l in (1, 3) else biasB
            bias_nxt = biasA if l == 0 else biasB if l == 1 else biasA

            # ---- (a) i-loop: bias chain ----
            # mish(x) = x*(1 - 2r), r = 1/(u^2+2u+2), u = e^x.  r is computed
            # as exp(-ln(w+2)) on the scalar LUT (exp+ln live in one table),
            # the final multiply as one AFFINE_MUL_REDUCE custom-DVE op.
            # Processed in half tiles [128, 2N] (head pairs) so the be-psum
            # can double-buffer (2 bufs x 2 banks).
            psum_bn = None
            psum_diff = [ps_d.tile([128, N], F32, tag=f"d{q}", name=f"pd{l}_{q}")
                         for q in range(2)]
            for i in range(R):
                half = (i % 2) * D
                for s in range(2):
                    psum_be = ps_be.tile([128, 2 * N], F32, tag="be")
                    if l == 0:
                        bias_t = bias0[i % 2]
                        if s == 0:
                            nc.vector.tensor_scalar(
                                bias_t[0:D, :], b0Tb[0:D, :],
                                b0L[:, i:i + 1], None, ALU.subtract)
                        for mm in range(2):
                            m = 2 * s + mm
                            nc.tensor.matmul(
                                psum_be[:, mm * N:(mm + 1) * N],
                                dWf0b[:, m * 128:(m + 1) * 128],
                                bias_t[:, :], start=True, stop=True)
                    else:
                        for mm in range(2):
                            m = 2 * s + mm
                            nc.tensor.matmul(
                                psum_be[:, mm * N:(mm + 1) * N],
                                diffW[l][half:half + D, m * 128:(m + 1) * 128],
                                bias_cur[half:half + D,
                                         (i // 2) * HHD:(i // 2) * HHD + HHD],
                                start=True, stop=True)
                    u_t = wpool.tile([128, 2 * N], BF16, tag="u",
                                     name=f"u{l}_{i}_{s}")
                    if l > 0 and diffb_nonzero:
                        xb = wpool.tile([128, 2 * N], BF16, tag="xb",
                                        name=f"xb{l}_{i}_{s}")
                        for mm in range(2):
                            m = 2 * s + mm
                            sl = slice(mm * N, (mm + 1) * N)
                            nc.scalar.activation(xb[:, sl], psum_be[:, sl],
                                                 AF.Identity,
                                                 bias=diffb[l][:, m:m + 1])
                        nc.scalar.activation(u_t[:], xb[:], AF.Exp)
                        x_src = xb
                    else:
                        nc.scalar.activation(u_t[:], psum_be[:], AF.Exp)
                        x_src = psum_be
                    # p = (u+1)^2 ; d = p+1 = u^2+2u+2 ; r ~= 1/d ;
                    # mish = (r*(-2)+1) * x   (one custom-DVE op)
                    p_t = wpool.tile([128, 2 * N], F32, tag="p",
                                     name=f"p{l}_{i}_{s}")
                    nc.scalar.activation(p_t[:], u_t[:], AF.Square,
                                         bias=onec[:])
                    d_t = wpool.tile([128, 2 * N], F32, tag="d",
                                     name=f"d{l}_{i}_{s}")
                    nc.vector.tensor_scalar(d_t[:], p_t[:], 1.0, None, ALU.add)
                    r_t = wpool.tile([128, 2 * N], F32, tag="r",
                                     name=f"r{l}_{i}_{s}")
                    nc.vector.reciprocal_approx_fast(out=r_t[:], in_=d_t[:])
                    mish_t = wpool.tile([128, 2 * N], BF16, tag="mish",
                                        name=f"mish{l}_{i}_{s}")
                    nc.vector._custom_dve(
                        AFFINE_MUL_REDUCE, out=mish_t[:], in0=r_t[:],
                        in1=x_src[:], s0=-2.0, s1=1.0)
                    sq_t = wpool.tile([128, 2 * N], BF16, tag="sq",
                                      name=f"sq{l}_{i}_{s}")
                    nc.scalar.activation(sq_t[:], mish_t[:], AF.Square)
                    # diffs accumulation (one-hot column matmuls): half s
                    # feeds head pair p == s
                    for hh in range(2):
                        col = hh * D + i
                        nc.tensor.matmul(
                            psum_diff[s][:],
                            strip[:, 127 - col:255 - col],
                            sq_t[:, hh * N:(hh + 1) * N],
                            start=(i == 0 and hh == 0),
                            stop=(i == R - 1 and hh == 1),
                            skip_group_check=True)
                    # next-layer bias (skip on last layer)
                    if l < L - 1:
                        if s == 0 and i % 2 == 0:
                            psum_bn = ps_bn.tile([128, HHD], F32, tag="bn",
                                                 name=f"bn{l}_{i}")
                        for mm in range(2):
                            m = 2 * s + mm
                            nc.tensor.matmul(
                                psum_bn[half:half + D, :],
                                boutW[l][:, m * 128 + half:m * 128 + half + D],
                                mish_t[:, mm * N:(mm + 1) * N],
                                start=(m == 0), stop=(m == 3),
                                tile_position=(0, half))
                if l < L - 1 and i % 2 == 1:
                    # mish on the accumulated [128, HHD] bias tile
                    bsl = slice((i // 2) * HHD, (i // 2) * HHD + HHD)
                    u2 = w2pool.tile([128, HHD], BF16, tag="u2",
                                     name=f"u2_{l}_{i}")
                    nc.scalar.activation(u2[:], psum_bn[:], AF.Exp,
                                         bias=boutb[:, l:l + 1])
                    p2 = w2pool.tile([128, HHD], F32, tag="p2",
                                     name=f"p2_{l}_{i}")
                    nc.scalar.activation(p2[:], u2[:], AF.Square,
                                         bias=onec[:])
                    d2 = w2pool.tile([128, HHD], F32, tag="d2",
                                     name=f"d2_{l}_{i}")
                    nc.vector.tensor_scalar(d2[:], p2[:], 1.0, None, ALU.add)
                    r2 = w2pool.tile([128, HHD], F32, tag="r2",
                                     name=f"r2_{l}_{i}")
                    nc.vector.reciprocal_approx_fast(out=r2[:], in_=d2[:])
                    tm2 = w2pool.tile([128, HHD], BF16, tag="tm2",
                                      name=f"tm2_{l}_{i}")
                    nc.vector.tensor_scalar(tm2[:], r2[:], -2.0, 1.0,
                                            ALU.mult, ALU.add)
                    nc.vector._custom_dve(
                        AFFINE_MUL_REDUCE, out=bias_nxt[:, bsl],
                        in0=psum_bn[:], in1=tm2[:], s0=1.0,
                        s1=boutb[:, l:l + 1])

            # ---- (b) sqrt window: diffs sqrt + LN -> x_l ----
            for p in range(2):
                nc.scalar.activation(diffs_s[p][:], psum_diff[p][:], AF.Sqrt)
            if l == n_layers - 1 and trunc != 0 and os.environ.get("KHALF") == "1":
                break
            if l > 0:
                for m in range(4):
                    dma(pre_all[:, m * D:(m + 1) * D],
                        gout[l - 1][m * 128:(m + 1) * 128, :])
            g_t, b_t = (ln1g, ln1b) if l == 0 else (ln2g, ln2b)
            insts = [(pre_all[:, m * D:(m + 1) * D], 128, g_t[:], b_t[:],
                      xfull[:, m * D:(m + 1) * D]) for m in range(4)]
            insts.append((resid_loc[:], R, g_t[0:R, :], b_t[0:R, :],
                          x_loc[:]))
            _ln_batch(nc, pools, insts, f"l{l}")
            if l == n_layers - 1 and trunc != 0 and \
                    int(os.environ.get("KPHASE", "9")) <= 0:
                break
            # transposes -> xT (augmented), xlocT (augmented)
            for m in range(4):
                pt = ps_x.tile([D, 128], F32, tag="x")
                nc.tensor.transpose(pt[:], xfull[:, m * D:(m + 1) * D], ident[:])
                nc.vector.tensor_copy(out=xT[0:D, m * 128:(m + 1) * 128],
                                      in_=pt[:])
            nc.gpsimd.memset(xT[D:D + 1, :], 1.0)
            ptl = ps_x.tile([D, R], F32, tag="x")
            nc.tensor.transpose(ptl[:], x_loc[:], ident[0:R, 0:R])
            nc.vector.tensor_copy(out=xlocT[0:D, :], in_=ptl[:])
            nc.gpsimd.memset(xlocT[D:D + 1, :], 1.0)
            if l == n_layers - 1 and trunc != 0 and \
                    int(os.environ.get("KPHASE", "9")) <= 1:
                break

            # ---- (c) qkv ----
            for h in range(H):
                base = h * 3 * HD
                # k^T for head h
                pk = ps_x.tile([HD, N], F32, tag="x")
                nc.tensor.matmul(pk[:], qkvW[l][:, base + HD:base + 2 * HD],
                                 xT[:], start=True, stop=True)
                nc.vector.tensor_copy(out=kT[:, h * N:(h + 1) * N], in_=pk[:])
                # q^T local rows
                pq = ps_x.tile([HD, R], F32, tag="x")
                nc.tensor.matmul(pq[:], qkvW[l][:, base:base + HD],
                                 xlocT[:], start=True, stop=True)
                nc.vector.tensor_copy(out=ql[:, h * R:(h + 1) * R], in_=pq[:])
                # v (untransposed) per token chunk
                for tc_ in range(4):
                    pv = ps_x.tile([128, HD], F32, tag="x")
                    nc.tensor.matmul(pv[:], xT[:, tc_ * 128:(tc_ + 1) * 128],
                                     qkvW[l][:, base + 2 * HD:base + 3 * HD],
                                     start=True, stop=True)
                    nc.vector.tensor_copy(
                        out=v_all[:, (h * 4 + tc_) * HD:(h * 4 + tc_ + 1) * HD],
                        in_=pv[:])

            if l == n_layers - 1 and trunc != 0 and \
                    int(os.environ.get("KPHASE", "9")) <= 2:
                break
            # ---- (d) attention per head ----
            for h in range(H):
                p, hh = h // 2, h % 2
                plg = ps_x.tile([R, N], F32, tag="x")
                nc.tensor.matmul(plg[:], ql[:, h * R:(h + 1) * R],
                                 kT[:, h * N:(h + 1) * N], start=True, stop=True)
                pre_sb = wpool.tile([R, N], BF16, tag="pre_sb")
                nc.vector.scalar_tensor_tensor(
                    out=pre_sb[:], in0=plg[:], scalar=SCALE,
                    in1=diffs_s[p][hh * R:(hh + 1) * R, :],
                    op0=ALU.mult, op1=ALU.add)
                nmax = statp.tile([R, 1], F32, tag="nmax")
                nc.vector.reduce_max(nmax[:], pre_sb[:], axis=AX.X, negate=True)
                esb = wpool.tile([R, N], BF16, tag="esb")
                sumexp = statp.tile([R, 1], F32, tag="sumexp")
                nc.scalar.activation(esb[:], pre_sb[:], AF.Exp,
                                     bias=nmax[:], accum_out=sumexp[:])
                rsum = statp.tile([R, 1], F32, tag="rsum")
                nc.vector.reciprocal(rsum[:], sumexp[:])
                att = wpool.tile([R, N], BF16, tag="att")
                nc.vector.tensor_scalar(att[:], esb[:], rsum[:], None, ALU.mult)
                attT = wpool.tile([128, 4 * R], BF16, tag="attT")
                for tc_ in range(4):
                    pat = ps_x.tile([128, R], BF16, tag="x")
                    nc.tensor.transpose(pat[:], att[:, tc_ * 128:(tc_ + 1) * 128],
                                        identb[0:R, 0:R])
                    nc.vector.tensor_copy(out=attT[:, tc_ * R:(tc_ + 1) * R],
                                          in_=pat[:])
                pvl = ps_x.tile([HD, R], F32, tag="x")
                for tc_ in range(4):
                    nc.tensor.matmul(
                        pvl[:],
                        v_all[:, (h * 4 + tc_) * HD:(h * 4 + tc_ + 1) * HD],
                        attT[:, tc_ * R:(tc_ + 1) * R],
                        start=(tc_ == 0), stop=(tc_ == 3))
                nc.vector.tensor_copy(out=va[:, h * R:(h + 1) * R], in_=pvl[:])

            if l == n_layers - 1 and trunc != 0 and \
                    int(os.environ.get("KPHASE", "9")) <= 3:
                break
            # ---- (e) output projection for local rows ----
            ptx = ps_x.tile([D, R], F32, tag="x")
            for h in range(H):
                nc.tensor.matmul(ptx[:], oW_sb[l][:, h * D:(h + 1) * D],
                                 va[:, h * R:(h + 1) * R],
                                 start=(h == 0), stop=(h == 3))
            tempxT = w64.tile([D, R], F32, tag="tempxT")
            nc.scalar.activation(tempxT[:], ptx[:], AF.Identity,
                                 bias=ob_sb[:, l:l + 1])
            # residual: resid_loc = x_loc + temp_x (untransposed)
            ptu = ps_x.tile([R, D], F32, tag="x")
            nc.tensor.transpose(ptu[:], tempxT[:], ident[0:D, 0:D])
            nc.vector.tensor_tensor(resid_loc[:], ptu[:], x_loc[:], ALU.add)

            # ---- (f) gather residual rows (layers 0-2) ----
            if l == n_layers - 1 and trunc != 0 and \
                    int(os.environ.get("KPHASE", "9")) <= 4:
                break
            if l < L - 1:
                nc.sync.dma_start(gin[l][:], resid_loc[:])
                nc.gpsimd.collective_compute(
                    "AllGather", ALU.bypass,
                    replica_groups=[list(range(NCORES))],
                    ins=[gin[l].opt()], outs=[gout[l].opt()])

        # ---------------- final: LN + out head on local rows ----------------
        if trunc > 1:
            _early_out()
        if trunc == 0:
            x4 = w64.tile([R, D], F32, tag="x4")
            _ln_batch(nc, pools, [(resid_loc[:], R, ln2g[0:R, :],
                                   ln2b[0:R, :], x4[:])], "fin")
            pxf = ps_x.tile([D, R], F32, tag="x")
            nc.tensor.transpose(pxf[:], x4[:], ident[0:R, 0:R])
            nc.vector.tensor_copy(out=xfT[0:D, :], in_=pxf[:])
            nc.gpsimd.memset(xfT[D:D + 1, :], 1.0)
            pout = ps_x.tile([R, 1], F32, tag="x")
            nc.tensor.matmul(pout[:], xfT[:], outW_sb[:], start=True, stop=True)
            osb = w64.tile([R, 1], F32, tag="osb")
            nc.vector.tensor_copy(osb[:], pout[:])
            nc.sync.dma_start(out_dram[:, :], osb[:])

    nc.compile()
    return nc


def _prep_inputs(inputs):
    f32 = np.float32

    def f(x):
        return np.ascontiguousarray(np.asarray(x), dtype=f32)

    nf = f(inputs["node_features"])
    amds = f(inputs["amds"])
    emb_W, emb_b = f(inputs["emb_W"]), f(inputs["emb_b"])
    bemb_W, bemb_b = f(inputs["bias_emb_W"]), f(inputs["bias_emb_b"])
    qkv_W, qkv_b = f(inputs["qkv_W"]), f(inputs["qkv_b"])
    diff_W, diff_b = f(inputs["diff_W"]), f(inputs["diff_b"])
    o_W, o_b = f(inputs["o_W"]), f(inputs["o_b"])
    bout_W, bout_b = f(inputs["bout_W"]), f(inputs["bout_b"])
    out_W, out_b = f(inputs["out_W"]), f(inputs["out_b"])
    ln1_g, ln1_b = f(inputs["ln1_g"]), f(inputs["ln1_b"])
    ln2_g, ln2_b = f(inputs["ln2_g"]), f(inputs["ln2_b"])

    ones_n = np.ones((1, N), f32)
    ones_r = np.ones((1, R), f32)
    com = {}
    com["nfT_aug"] = np.ascontiguousarray(
        np.concatenate([nf.T, ones_n], 0))
    com["amdsT_aug"] = np.ascontiguousarray(
        np.concatenate([amds.T, ones_n], 0))
    com["embW_aug"] = np.concatenate([emb_W, emb_b[None, :]], 0)
    com["bembW_aug"] = np.concatenate([bemb_W, bemb_b[None, :]], 0)
    com["qkvW_aug"] = np.ascontiguousarray(
        np.concatenate([qkv_W, qkv_b[:, None, :]], 1)).astype(NP_BF16)
    com["dWf0_aug"] = np.concatenate([diff_W[0], diff_b[0][None, :]], 0)
    com["diffW_dup"] = np.ascontiguousarray(
        np.concatenate([diff_W, diff_W], 1)).astype(NP_BF16)
    com["diffb_cols"] = np.ascontiguousarray(
        diff_b.reshape(L, H, HD).transpose(0, 2, 1))
    bwd = np.zeros((L, HD, 8 * D), f32)
    for l in range(L):
        for h in range(H):
            chunk = bout_W[l, h * HD:(h + 1) * HD, :]  # [128, 64]
            bwd[l, :, h * 2 * D:h * 2 * D + D] = chunk
            bwd[l, :, h * 2 * D + D:h * 2 * D + 2 * D] = chunk
    com["boutW_dup"] = bwd.astype(NP_BF16)
    com["boutb2"] = np.ascontiguousarray(
        np.tile(bout_b, (1, 2)).T)  # [128, L]
    com["oW"] = o_W.astype(NP_BF16)
    com["ob_cols"] = np.ascontiguousarray(o_b.T)
    com["outW_aug"] = np.concatenate([out_W, out_b[None, :]], 0)
    com["ln1g_t"] = np.tile(ln1_g[None, :], (HD, 1))
    com["ln1b_t"] = np.tile(ln1_b[None, :], (HD, 1))
    com["ln2g_t"] = np.tile(ln2_g[None, :], (HD, 1))
    com["ln2b_t"] = np.tile(ln2_b[None, :], (HD, 1))
    strip = np.zeros((HD, 255), f32)
    strip[:, 127] = 1.0
    com["strip"] = strip.astype(NP_BF16)

    in_maps = []
    for c in range(NCORES):
        m = dict(com)
        m["nfT_loc"] = np.ascontiguousarray(
            np.concatenate([nf.T[:, c * R:(c + 1) * R], ones_r], 0))
        m["amdsT_loc"] = np.ascontiguousarray(
            np.concatenate([amds.T[:, c * R:(c + 1) * R], ones_r], 0))
        in_maps.append(m)
    diffb_nonzero = bool(np.any(diff_b != 0.0))
    return in_maps, diffb_nonzero


_LAST_RESULTS = None


def kernel(**inputs) -> np.ndarray:
    global _LAST_RESULTS
    in_maps, diffb_nonzero = _prep_inputs(inputs)
    trunc = int(os.environ.get("KTRUNC", "0"))
    nc = _build(diffb_nonzero, trunc)
    trace = bool(int(os.environ.get("KERNEL_TRACE", "0")))
    try:
        res = bass_utils.run_bass_kernel_spmd(
            nc, in_maps, core_ids=list(range(NCORES)), trace=trace)
    except ModuleNotFoundError:
        res = bass_utils.run_bass_kernel_spmd(
            nc, in_maps, core_ids=list(range(NCORES)), trace=False)
    _LAST_RESULTS = res
    out = np.concatenate(
        [res.results[c]["out_loc"] for c in range(NCORES)], axis=0)
    return out.astype(np.float32)


if __name__ == "__main__":
    rng = np.random.default_rng(0)
    dummy = {
        "node_features": rng.standard_normal((N, FB), dtype=np.float32),
        "amds": rng.random((N, K), dtype=np.float32),
        "emb_W": rng.standard_normal((FB, D), dtype=np.float32) / 16,
        "emb_b": np.zeros((D,), np.float32),
        "bias_emb_W": rng.standard_normal((K, D), dtype=np.float32) / 10,
        "bias_emb_b": np.zeros((D,), np.float32),
        "ln1_g": np.ones((D,), np.float32),
        "ln1_b": np.zeros((D,), np.float32),
        "ln2_g": np.ones((D,), np.float32),
        "ln2_b": np.zeros((D,), np.float32),
        "qkv_W": rng.standard_normal((L, D, 3 * HHD), dtype=np.float32) / 8,
        "qkv_b": np.zeros((L, 3 * HHD), np.float32),
        "diff_W": rng.standard_normal((L, D, HHD), dtype=np.float32) / 8,
        "diff_b": np.zeros((L, HHD), np.float32),
        "o_W": rng.standard_normal((L, HHD, D), dtype=np.float32) / 22,
        "o_b": np.zeros((L, D), np.float32),
        "bout_W": rng.standard_normal((L, HHD, D), dtype=np.float32) / 22,
        "bout_b": np.zeros((L, D), np.float32),
        "out_W": rng.standard_normal((D, 1), dtype=np.float32) / 8,
        "out_b": np.zeros((1,), np.float32),
    }
    out = kernel(**dummy)
    print("kernel output shape:", out.shape, "first:", out[:4, 0])



# revision 29
# speedup vs baseline: 1.2690x; 1.0603x over previous
"""Trainium2 Bass kernel for nn_CrAKN (dense transformer with pairwise bias chain).

Sharding: rows of the N=512 crystal dimension are split across 8 cores
(64 rows each). Each core computes its [64, N, 512] bias-chain slice and its
64 attention rows; per layer the updated residual rows are AllGathered so
every core can form the full k/v for the next layer.

Self-contained: hardcodes all shapes; builds one SPMD Bass program and runs
it via run_bass_kernel_spmd on cores 0-7.
"""

import os
import sys
import functools
from contextlib import ExitStack

import numpy as np

sys.path.insert(0, "/opt/trn_rl_repo")

import concourse.bass as bass  # noqa: E402
import concourse.bacc as bacc  # noqa: E402
import concourse.tile as tile  # noqa: E402
import concourse.mybir as mybir  # noqa: E402
import concourse.bass_utils as bass_utils  # noqa: E402
from concourse.masks import make_identity  # noqa: E402
from concourse.dve_ops import AFFINE_MUL_REDUCE  # noqa: E402

F32 = mybir.dt.float32
BF16 = mybir.dt.bfloat16
NP_BF16 = mybir.dt.np(BF16)

AF = mybir.ActivationFunctionType
ALU = mybir.AluOpType
AX = mybir.AxisListType

N, FB, D, H, HD, L, K = 512, 256, 64, 128, 4, 4, 100
H, HD = 4, 128
HHD = H * HD  # 512
NCORES = 8
R = N // NCORES  # 64 rows per core
EPS = 1e-5
SCALE = 1.0 / float(np.sqrt(HD))


def _ln_batch(nc, pools, insts, tag):
    """Batched LayerNorm: insts = list of (in_ap, parts, g_ap, b_ap, out_ap).
    All variances collect into one [128, K] tile so a single scalar Sqrt
    serves every instance (avoids act-table thrashing)."""
    stat = pools["stat"]
    work = pools["work64"]
    K = len(insts)
    varb = stat.tile([128, K], F32, tag="ln_varb", name=f"varb_{tag}")
    cens = []
    for k, (in_ap, parts, g_ap, b_ap, out_ap) in enumerate(insts):
        ssum = stat.tile([parts, 1], F32, tag="ln_sum")
        nc.vector.reduce_sum(ssum[:], in_ap, axis=AX.X)
        mu = stat.tile([parts, 1], F32, tag="ln_mu")
        nc.vector.tensor_scalar(mu[:], ssum[:], 1.0 / D, None, ALU.mult)
        cen = work.tile([parts, D], F32, tag=f"ln_cen{k}",
                        name=f"cen_{tag}_{k}")
        nc.vector.tensor_scalar(cen[:], in_ap, mu[:], None, ALU.subtract)
        vscr = work.tile([parts, D], F32, tag="ln_vscr")
        nc.vector.tensor_tensor(vscr[:], cen[:], cen[:], ALU.mult)
        nc.vector.reduce_sum(varb[0:parts, k:k + 1], vscr[:], axis=AX.X)
        cens.append(cen)
    sd = stat.tile([128, K], F32, tag="ln_sd", name=f"sd_{tag}")
    nc.scalar.activation(sd[:], varb[:], AF.Sqrt, scale=1.0 / D,
                         bias=pools["eps"])
    rsb = stat.tile([128, K], F32, tag="ln_rs", name=f"rs_{tag}")
    nc.vector.reciprocal(rsb[:], sd[:])
    for k, (in_ap, parts, g_ap, b_ap, out_ap) in enumerate(insts):
        xg = work.tile([parts, D], F32, tag="ln_xg")
        nc.vector.scalar_tensor_tensor(
            out=xg[:], in0=cens[k][:], scalar=rsb[0:parts, k:k + 1],
            in1=g_ap, op0=ALU.mult, op1=ALU.mult)
        nc.vector.tensor_tensor(out_ap, xg[:], b_ap, ALU.add)


@functools.lru_cache(maxsize=4)
def _build(diffb_nonzero: bool, trunc: int = 0):
    nc = bacc.Bacc("TRN2", target_bir_lowering=False, debug=False,
                   enable_asserts=False, num_devices=NCORES)

    def din(name, shape, dt=F32):
        return nc.dram_tensor(name, list(shape), dt, kind="ExternalInput").ap()

    nfT_aug = din("nfT_aug", (FB + 1, N))
    nfT_loc = din("nfT_loc", (FB + 1, R))
    amdsT_aug = din("amdsT_aug", (K + 1, N))
    amdsT_loc = din("amdsT_loc", (K + 1, R))
    embW_aug = din("embW_aug", (FB + 1, D))
    bembW_aug = din("bembW_aug", (K + 1, D))
    qkvW_aug_d = din("qkvW_aug", (L, D + 1, 3 * HHD), BF16)
    dWf0_aug_d = din("dWf0_aug", (D + 1, HHD))
    diffW_dup_d = din("diffW_dup", (L, 2 * D, HHD), BF16)
    diffb_d = din("diffb_cols", (L, HD, H))
    boutW_dup_d = din("boutW_dup", (L, HD, 8 * D), BF16)
    boutb_d = din("boutb2", (HD, L))
    oW_d = din("oW", (L, HHD, D), BF16)
    ob_d = din("ob_cols", (D, L))
    outW_aug_d = din("outW_aug", (D + 1, 1))
    ln1g_d = din("ln1g_t", (HD, D))
    ln1b_d = din("ln1b_t", (HD, D))
    ln2g_d = din("ln2g_t", (HD, D))
    ln2b_d = din("ln2b_t", (HD, D))
    strip_d = din("strip", (HD, 255), BF16)

    out_dram = nc.dram_tensor("out_loc", [R, 1], F32, kind="ExternalOutput").ap()

    with nc.allow_low_precision(reason="bf16 mish rational chain"), \
         tile.TileContext(nc) as tc, ExitStack() as ctx:
        cpool = ctx.enter_context(tc.tile_pool(name="const", bufs=1))
        ppool = ctx.enter_context(tc.tile_pool(name="persist", bufs=1))
        wpool = ctx.enter_context(tc.tile_pool(name="work", bufs=2))
        w2pool = ctx.enter_context(tc.tile_pool(name="work2", bufs=2))
        w64 = ctx.enter_context(tc.tile_pool(name="work64", bufs=2))
        statp = ctx.enter_context(tc.tile_pool(name="stat", bufs=4))
        mpool = ctx.enter_context(tc.tile_pool(name="mish", bufs=4))
        ps_be = ctx.enter_context(tc.tile_pool(name="ps_be", bufs=2, space="PSUM"))
        ps_d = ctx.enter_context(tc.tile_pool(name="ps_d", bufs=1, space="PSUM"))
        ps_bn = ctx.enter_context(tc.tile_pool(name="ps_bn", bufs=1, space="PSUM"))
        ps_x = ctx.enter_context(tc.tile_pool(name="ps_x", bufs=1, space="PSUM"))
        dram = ctx.enter_context(tc.tile_pool(name="dram", bufs=1, space="DRAM"))
        pools = {"stat": statp, "work64": w64}

        dma = nc.sync.dma_start

        # ---------------- constants into SBUF ----------------
        def cload(name, shape, src_ap, dt=F32):
            t = cpool.tile(list(shape), dt, tag=name, name=name)
            dma(t[:], src_ap)
            return t

        # node features transposed (3 K-chunks: 128/128/1)
        nfT0 = cload("nfT0", [128, N], nfT_aug[0:128, :])
        nfT1 = cload("nfT1", [128, N], nfT_aug[128:256, :])
        nfT2 = cload("nfT2", [1, N], nfT_aug[256:257, :])
        nfl0 = cload("nfl0", [128, R], nfT_loc[0:128, :])
        nfl1 = cload("nfl1", [128, R], nfT_loc[128:256, :])
        nfl2 = cload("nfl2", [1, R], nfT_loc[256:257, :])
        embW0 = cload("embW0", [128, D], embW_aug[0:128, :])
        embW1 = cload("embW1", [128, D], embW_aug[128:256, :])
        embW2 = cload("embW2", [1, D], embW_aug[256:257, :])
        amds_sb = cload("amds_sb", [K + 1, N], amdsT_aug[:, :])
        amdl_sb = cload("amdl_sb", [K + 1, R], amdsT_loc[:, :])
        bembW = cload("bembW", [K + 1, D], bembW_aug[:, :])
        dWf0 = cload("dWf0", [D + 1, HHD], dWf0_aug_d[:, :])
        qkvW = [cload(f"qkvW{l}", [D + 1, 3 * HHD], qkvW_aug_d[l, :, :], BF16)
                for l in range(L)]
        diffW = [cload(f"diffW{l}", [2 * D, HHD], diffW_dup_d[l, :, :], BF16)
                 for l in range(1, L)]
        diffW = [None] + diffW
        diffb = [cload(f"diffb{l}", [HD, H], diffb_d[l, :, :])
                 for l in range(L)] if diffb_nonzero else None
        boutW = [cload(f"boutW{l}", [HD, 8 * D], boutW_dup_d[l, :, :], BF16)
                 for l in range(L - 1)]
        boutb = cload("boutb", [HD, L], boutb_d[:, :])
        oW_sb = []
        for l in range(L):
            t = cpool.tile([HD, H * D], BF16, tag=f"oW{l}", name=f"oW{l}")
            for h in range(H):
                dma(t[:, h * D:(h + 1) * D], oW_d[l, h * HD:(h + 1) * HD, :])
            oW_sb.append(t)
        ob_sb = cload("ob_sb", [D, L], ob_d[:, :])
        outW_sb = cload("outW_sb", [D + 1, 1], outW_aug_d[:, :])
        ln1g = cload("ln1g", [HD, D], ln1g_d[:, :])
        ln1b = cload("ln1b", [HD, D], ln1b_d[:, :])
        ln2g = cload("ln2g", [HD, D], ln2g_d[:, :])
        ln2b = cload("ln2b", [HD, D], ln2b_d[:, :])
        strip = cload("strip", [HD, 255], strip_d[:, :], BF16)

        ident = cpool.tile([128, 128], F32, tag="ident")
        make_identity(nc, ident[:])
        identb = cpool.tile([128, 128], BF16, tag="identb")
        make_identity(nc, identb[:])
        epsc = cpool.tile([128, 1], F32, tag="epsc")
        nc.gpsimd.memset(epsc[:], EPS)
        pools["eps"] = epsc
        onec = cpool.tile([128, 1], F32, tag="onec")
        nc.gpsimd.memset(onec[:], 1.0)

        # ---------------- persistent tiles ----------------
        biasA = ppool.tile([128, R * HHD // 2], BF16, tag="biasA")
        biasB = ppool.tile([128, R * HHD // 2], BF16, tag="biasB")
        b0L = ppool.tile([D, R], F32, tag="b0L")
        b0Tb = ppool.tile([D + 1, N], BF16, tag="b0Tb")
        bias0 = [ppool.tile([D + 1, N], BF16, tag=f"bias0_{par}",
                            name=f"bias0_{par}") for par in range(2)]
        xT = ppool.tile([D + 1, N], BF16, tag="xT")
        xlocT = ppool.tile([D + 1, R], BF16, tag="xlocT")
        x_loc = ppool.tile([R, D], F32, tag="x_loc")
        resid_loc = ppool.tile([R, D], F32, tag="resid_loc")
        pre_all = ppool.tile([128, 4 * D], F32, tag="pre_all")
        xfull = ppool.tile([128, 4 * D], F32, tag="xfull")
        kT = ppool.tile([HD, H * N], BF16, tag="kT")
        v_all = ppool.tile([128, H * HD * 4 // 4 * 4], BF16, tag="v_all")  # [128, 2048]
        ql = ppool.tile([HD, H * R], BF16, tag="ql")
        va = ppool.tile([HD, H * R], BF16, tag="va")
        diffs_s = [ppool.tile([128, N], F32, tag=f"diffs{p}", name=f"diffs{p}")
                   for p in range(2)]
        xfT = ppool.tile([D + 1, R], F32, tag="xfT")

        # collective bounce buffers
        gin = [dram.tile([R, D], F32, tag=f"gin{l}", name=f"gin{l}")
               for l in range(L - 1)]
        gout = [dram.tile([N, D], F32, tag=f"gout{l}", name=f"gout{l}")
                for l in range(L - 1)]

        # ---------------- head: h, b0, G ----------------
        # full pre-activation h rows -> pre_all ([128, 64] x 4 tiles)
        for m in range(4):
            ph = ps_x.tile([128, D], F32, tag="x")
            nc.tensor.matmul(ph[:], nfT0[:, m * 128:(m + 1) * 128], embW0[:],
                             start=True, stop=False)
            nc.tensor.matmul(ph[:], nfT1[:, m * 128:(m + 1) * 128], embW1[:],
                             start=False, stop=False)
            nc.tensor.matmul(ph[:], nfT2[:, m * 128:(m + 1) * 128], embW2[:],
                             start=False, stop=True)
            nc.vector.tensor_copy(out=pre_all[:, m * D:(m + 1) * D], in_=ph[:])
        # local pre-activation rows -> resid_loc
        pl = ps_x.tile([R, D], F32, tag="x")
        nc.tensor.matmul(pl[:], nfl0[:], embW0[:], start=True, stop=False)
        nc.tensor.matmul(pl[:], nfl1[:], embW1[:], start=False, stop=False)
        nc.tensor.matmul(pl[:], nfl2[:], embW2[:], start=False, stop=True)
        nc.vector.tensor_copy(resid_loc[:], pl[:])
        # b0 transposed (bf16, augmented ones row) and local columns
        pb = ps_x.tile([D, N], F32, tag="x")
        nc.tensor.matmul(pb[:], bembW[:], amds_sb[:], start=True, stop=True)
        nc.vector.tensor_copy(out=b0Tb[0:D, :], in_=pb[:])
        nc.gpsimd.memset(b0Tb[D:D + 1, :], 1.0)
        pbl = ps_x.tile([D, R], F32, tag="x")
        nc.tensor.matmul(pbl[:], bembW[:], amdl_sb[:], start=True, stop=True)
        nc.vector.tensor_copy(b0L[:], pbl[:])
        # bf16 copy of the augmented diff_W[0] for the l=0 per-row matmuls
        dWf0b = cpool.tile([D + 1, HHD], BF16, tag="dWf0b", name="dWf0b")
        nc.vector.tensor_copy(out=dWf0b[:], in_=dWf0[:])
        for par in range(2):
            nc.gpsimd.memset(bias0[par][D:D + 1, :], 1.0)

        def _early_out():
            osb_e = w64.tile([R, 1], F32, tag="osb", name="osb_e")
            nc.vector.tensor_copy(osb_e[:], resid_loc[:, 0:1])
            nc.sync.dma_start(out_dram[:, :], osb_e[:])

        if trunc == 1:
            _early_out()
        n_layers = L if trunc == 0 else min(L, trunc - 1)

        # ---------------- layers ----------------
        for l in range(n_layers):
            bias_cur = biasA if l in (1, 3) else biasB
            bias_nxt = biasA if l == 0 else biasB if l == 1 else biasA

            # ---- (a) i-loop: bias chain ----
            # mish(x) = x*(1 - 2r), r = 1/(u^2+2u+2), u = e^x.  r is computed
            # as exp(-ln(w+2)) on the scalar LUT (exp+ln live in one table),
            # the final multiply as one AFFINE_MUL_REDUCE custom-DVE op.
            # Processed in half tiles [128, 2N] (head pairs) so the be-psum
            # can double-buffer (2 bufs x 2 banks).
            psum_bn = None
            psum_diff = [ps_d.tile([128, N], F32, tag=f"d{q}", name=f"pd{l}_{q}")
                         for q in range(2)]
            mish_hist = {}
            for i in range(R):
                half = (i % 2) * D
                for s in range(2):
                    psum_be = ps_be.tile([128, 2 * N], F32, tag="be")
                    if l == 0:
                        bias_t = bias0[i % 2]
                        if s == 0:
                            nc.vector.tensor_scalar(
                                bias_t[0:D, :], b0Tb[0:D, :],
                                b0L[:, i:i + 1], None, ALU.subtract)
                        for mm in range(2):
                            m = 2 * s + mm
                            nc.tensor.matmul(
                                psum_be[:, mm * N:(mm + 1) * N],
                                dWf0b[:, m * 128:(m + 1) * 128],
                                bias_t[:, :], start=True, stop=True)
                    else:
                        for mm in range(2):
                            m = 2 * s + mm
                            nc.tensor.matmul(
                                psum_be[:, mm * N:(mm + 1) * N],
                                diffW[l][half:half + D, m * 128:(m + 1) * 128],
                                bias_cur[half:half + D,
                                         (i // 2) * HHD:(i // 2) * HHD + HHD],
                                start=True, stop=True)
                    u_t = wpool.tile([128, 2 * N], BF16, tag="u",
                                     name=f"u{l}_{i}_{s}")
                    if l > 0 and diffb_nonzero:
                        xb = wpool.tile([128, 2 * N], BF16, tag="xb",
                                        name=f"xb{l}_{i}_{s}")
                        for mm in range(2):
                            m = 2 * s + mm
                            sl = slice(mm * N, (mm + 1) * N)
                            nc.scalar.activation(xb[:, sl], psum_be[:, sl],
                                                 AF.Identity,
                                                 bias=diffb[l][:, m:m + 1])
                        nc.scalar.activation(u_t[:], xb[:], AF.Exp)
                        x_src = xb
                    else:
                        nc.scalar.activation(u_t[:], psum_be[:], AF.Exp)
                        x_src = psum_be
                    # p = (u+1)^2 ; d = p+1 = u^2+2u+2 ; r ~= 1/d ;
                    # mish = (r*(-2)+1) * x   (one custom-DVE op)
                    p_t = wpool.tile([128, 2 * N], F32, tag="p",
                                     name=f"p{l}_{i}_{s}")
                    nc.scalar.activation(p_t[:], u_t[:], AF.Square,
                                         bias=onec[:])
                    d_t = wpool.tile([128, 2 * N], F32, tag="d",
                                     name=f"d{l}_{i}_{s}")
                    nc.vector.tensor_scalar(d_t[:], p_t[:], 1.0, None, ALU.add)
                    r_t = wpool.tile([128, 2 * N], F32, tag="r",
                                     name=f"r{l}_{i}_{s}")
                    nc.vector.reciprocal_approx_fast(out=r_t[:], in_=d_t[:])
                    mish_t = mpool.tile([128, 2 * N], BF16, tag="mish",
                                        name=f"mish{l}_{i}_{s}")
                    nc.vector._custom_dve(
                        AFFINE_MUL_REDUCE, out=mish_t[:], in0=r_t[:],
                        in1=x_src[:], s0=-2.0, s1=1.0)
                    sq_t = wpool.tile([128, 2 * N], BF16, tag="sq",
                                      name=f"sq{l}_{i}_{s}")
                    nc.scalar.activation(sq_t[:], mish_t[:], AF.Square)
                    # diffs accumulation (one-hot column matmuls): half s
                    # feeds head pair p == s
                    for hh in range(2):
                        col = hh * D + i
                        nc.tensor.matmul(
                            psum_diff[s][:],
                            strip[:, 127 - col:255 - col],
                            sq_t[:, hh * N:(hh + 1) * N],
                            start=(i == 0 and hh == 0),
                            stop=(i == R - 1 and hh == 1),
                            skip_group_check=True)
                    if s == 1:
                        mish_hist[i] = (mish_hist.get(i, (None,))[0], mish_t)
                    else:
                        mish_hist[i] = (mish_t,)
                # next-layer bias matmuls, delayed one iteration so the
                # single psum_bn bank has slack for its mish chain
                if l < L - 1:
                    for j in ([i - 1] if i < R - 1 else [i - 1, i]):
                        if j < 0:
                            continue
                        jhalf = (j % 2) * D
                        if j % 2 == 0:
                            psum_bn = ps_bn.tile([128, HHD], F32, tag="bn",
                                                 name=f"bn{l}_{j}")
                        for m in range(4):
                            nc.tensor.matmul(
                                psum_bn[jhalf:jhalf + D, :],
                                boutW[l][:, m * 128 + jhalf:
                                         m * 128 + jhalf + D],
                                mish_hist[j][m // 2][:,
                                                     (m % 2) * N:
                                                     (m % 2 + 1) * N],
                                start=(m == 0), stop=(m == 3),
                                tile_position=(0, jhalf))
                        if j % 2 == 1:
                            bsl = slice((j // 2) * HHD, (j // 2) * HHD + HHD)
                            u2 = w2pool.tile([128, HHD], BF16, tag="u2",
                                             name=f"u2_{l}_{j}")
                            nc.scalar.activation(u2[:], psum_bn[:], AF.Exp,
                                                 bias=boutb[:, l:l + 1])
                            p2 = w2pool.tile([128, HHD], F32, tag="p2",
                                             name=f"p2_{l}_{j}")
                            nc.scalar.activation(p2[:], u2[:], AF.Square,
                                                 bias=onec[:])
                            d2 = w2pool.tile([128, HHD], F32, tag="d2",
                                             name=f"d2_{l}_{j}")
                            nc.vector.tensor_scalar(d2[:], p2[:], 1.0, None,
                                                    ALU.add)
                            r2 = w2pool.tile([128, HHD], F32, tag="r2",
                                             name=f"r2_{l}_{j}")
                            nc.vector.reciprocal_approx_fast(out=r2[:],
                                                             in_=d2[:])
                            tm2 = w2pool.tile([128, HHD], BF16, tag="tm2",
                                              name=f"tm2_{l}_{j}")
                            nc.vector.tensor_scalar(tm2[:], r2[:], -2.0, 1.0,
                                                    ALU.mult, ALU.add)
                            nc.vector._custom_dve(
                                AFFINE_MUL_REDUCE, out=bias_nxt[:, bsl],
                                in0=psum_bn[:], in1=tm2[:], s0=1.0,
                                s1=boutb[:, l:l + 1])

            # ---- (b) sqrt window: diffs sqrt + LN -> x_l ----
            for p in range(2):
                nc.scalar.activation(diffs_s[p][:], psum_diff[p][:], AF.Sqrt)
            if l == n_layers - 1 and trunc != 0 and os.environ.get("KHALF") == "1":
                break
            if l > 0:
                for m in range(4):
                    dma(pre_all[:, m * D:(m + 1) * D],
                        gout[l - 1][m * 128:(m + 1) * 128, :])
            g_t, b_t = (ln1g, ln1b) if l == 0 else (ln2g, ln2b)
            insts = [(pre_all[:, m * D:(m + 1) * D], 128, g_t[:], b_t[:],
                      xfull[:, m * D:(m + 1) * D]) for m in range(4)]
            insts.append((resid_loc[:], R, g_t[0:R, :], b_t[0:R, :],
                          x_loc[:]))
            _ln_batch(nc, pools, insts, f"l{l}")
            if l == n_layers - 1 and trunc != 0 and \
                    int(os.environ.get("KPHASE", "9")) <= 0:
                break
            # transposes -> xT (augmented), xlocT (augmented)
            for m in range(4):
                pt = ps_x.tile([D, 128], F32, tag="x")
                nc.tensor.transpose(pt[:], xfull[:, m * D:(m + 1) * D], ident[:])
                nc.vector.tensor_copy(out=xT[0:D, m * 128:(m + 1) * 128],
                                      in_=pt[:])
            nc.gpsimd.memset(xT[D:D + 1, :], 1.0)
            ptl = ps_x.tile([D, R], F32, tag="x")
            nc.tensor.transpose(ptl[:], x_loc[:], ident[0:R, 0:R])
            nc.vector.tensor_copy(out=xlocT[0:D, :], in_=ptl[:])
            nc.gpsimd.memset(xlocT[D:D + 1, :], 1.0)
            if l == n_layers - 1 and trunc != 0 and \
                    int(os.environ.get("KPHASE", "9")) <= 1:
                break

            # ---- (c) qkv ----
            for h in range(H):
                base = h * 3 * HD
                # k^T for head h
                pk = ps_x.tile([HD, N], F32, tag="x")
                nc.tensor.matmul(pk[:], qkvW[l][:, base + HD:base + 2 * HD],
                                 xT[:], start=True, stop=True)
                nc.vector.tensor_copy(out=kT[:, h * N:(h + 1) * N], in_=pk[:])
                # q^T local rows
                pq = ps_x.tile([HD, R], F32, tag="x")
                nc.tensor.matmul(pq[:], qkvW[l][:, base:base + HD],
                                 xlocT[:], start=True, stop=True)
                nc.vector.tensor_copy(out=ql[:, h * R:(h + 1) * R], in_=pq[:])
                # v (untransposed) per token chunk
                for tc_ in range(4):
                    pv = ps_x.tile([128, HD], F32, tag="x")
                    nc.tensor.matmul(pv[:], xT[:, tc_ * 128:(tc_ + 1) * 128],
                                     qkvW[l][:, base + 2 * HD:base + 3 * HD],
                                     start=True, stop=True)
                    nc.vector.tensor_copy(
                        out=v_all[:, (h * 4 + tc_) * HD:(h * 4 + tc_ + 1) * HD],
                        in_=pv[:])

            if l == n_layers - 1 and trunc != 0 and \
                    int(os.environ.get("KPHASE", "9")) <= 2:
                break
            # ---- (d) attention per head ----
            for h in range(H):
                p, hh = h // 2, h % 2
                plg = ps_x.tile([R, N], F32, tag="x")
                nc.tensor.matmul(plg[:], ql[:, h * R:(h + 1) * R],
                                 kT[:, h * N:(h + 1) * N], start=True, stop=True)
                pre_sb = wpool.tile([R, N], BF16, tag="pre_sb")
                nc.vector.scalar_tensor_tensor(
                    out=pre_sb[:], in0=plg[:], scalar=SCALE,
                    in1=diffs_s[p][hh * R:(hh + 1) * R, :],
                    op0=ALU.mult, op1=ALU.add)
                nmax = statp.tile([R, 1], F32, tag="nmax")
                nc.vector.reduce_max(nmax[:], pre_sb[:], axis=AX.X, negate=True)
                esb = wpool.tile([R, N], BF16, tag="esb")
                sumexp = statp.tile([R, 1], F32, tag="sumexp")
                nc.scalar.activation(esb[:], pre_sb[:], AF.Exp,
                                     bias=nmax[:], accum_out=sumexp[:])
                rsum = statp.tile([R, 1], F32, tag="rsum")
                nc.vector.reciprocal(rsum[:], sumexp[:])
                att = wpool.tile([R, N], BF16, tag="att")
                nc.vector.tensor_scalar(att[:], esb[:], rsum[:], None, ALU.mult)
                attT = wpool.tile([128, 4 * R], BF16, tag="attT")
                for tc_ in range(4):
                    pat = ps_x.tile([128, R], BF16, tag="x")
                    nc.tensor.transpose(pat[:], att[:, tc_ * 128:(tc_ + 1) * 128],
                                        identb[0:R, 0:R])
                    nc.vector.tensor_copy(out=attT[:, tc_ * R:(tc_ + 1) * R],
                                          in_=pat[:])
                pvl = ps_x.tile([HD, R], F32, tag="x")
                for tc_ in range(4):
                    nc.tensor.matmul(
                        pvl[:],
                        v_all[:, (h * 4 + tc_) * HD:(h * 4 + tc_ + 1) * HD],
                        attT[:, tc_ * R:(tc_ + 1) * R],
                        start=(tc_ == 0), stop=(tc_ == 3))
                nc.vector.tensor_copy(out=va[:, h * R:(h + 1) * R], in_=pvl[:])

            if l == n_layers - 1 and trunc != 0 and \
                    int(os.environ.get("KPHASE", "9")) <= 3:
                break
            # ---- (e) output projection for local rows ----
            ptx = ps_x.tile([D, R], F32, tag="x")
            for h in range(H):
                nc.tensor.matmul(ptx[:], oW_sb[l][:, h * D:(h + 1) * D],
                                 va[:, h * R:(h + 1) * R],
                                 start=(h == 0), stop=(h == 3))
            tempxT = w64.tile([D, R], F32, tag="tempxT")
            nc.scalar.activation(tempxT[:], ptx[:], AF.Identity,
                                 bias=ob_sb[:, l:l + 1])
            # residual: resid_loc = x_loc + temp_x (untransposed)
            ptu = ps_x.tile([R, D], F32, tag="x")
            nc.tensor.transpose(ptu[:], tempxT[:], ident[0:D, 0:D])
            nc.vector.tensor_tensor(resid_loc[:], ptu[:], x_loc[:], ALU.add)

            # ---- (f) gather residual rows (layers 0-2) ----
            if l == n_layers - 1 and trunc != 0 and \
                    int(os.environ.get("KPHASE", "9")) <= 4:
                break
            if l < L - 1:
                nc.sync.dma_start(gin[l][:], resid_loc[:])
                nc.gpsimd.collective_compute(
                    "AllGather", ALU.bypass,
                    replica_groups=[list(range(NCORES))],
                    ins=[gin[l].opt()], outs=[gout[l].opt()])

        # ---------------- final: LN + out head on local rows ----------------
        if trunc > 1:
            _early_out()
        if trunc == 0:
            x4 = w64.tile([R, D], F32, tag="x4")
            _ln_batch(nc, pools, [(resid_loc[:], R, ln2g[0:R, :],
                                   ln2b[0:R, :], x4[:])], "fin")
            pxf = ps_x.tile([D, R], F32, tag="x")
            nc.tensor.transpose(pxf[:], x4[:], ident[0:R, 0:R])
            nc.vector.tensor_copy(out=xfT[0:D, :], in_=pxf[:])
            nc.gpsimd.memset(xfT[D:D + 1, :], 1.0)
            pout = ps_x.tile([R, 1], F32, tag="x")
            nc.tensor.matmul(pout[:], xfT[:], outW_sb[:], start=True, stop=True)
            osb = w64.tile([R, 1], F32, tag="osb")
            nc.vector.tensor_copy(osb[:], pout[:])
            nc.sync.dma_start(out_dram[:, :], osb[:])

    nc.compile()
    return nc


def _prep_inputs(inputs):
    f32 = np.float32

    def f(x):
        return np.ascontiguousarray(np.asarray(x), dtype=f32)

    nf = f(inputs["node_features"])
    amds = f(inputs["amds"])
    emb_W, emb_b = f(inputs["emb_W"]), f(inputs["emb_b"])
    bemb_W, bemb_b = f(inputs["bias_emb_W"]), f(inputs["bias_emb_b"])
    qkv_W, qkv_b = f(inputs["qkv_W"]), f(inputs["qkv_b"])
    diff_W, diff_b = f(inputs["diff_W"]), f(inputs["diff_b"])
    o_W, o_b = f(inputs["o_W"]), f(inputs["o_b"])
    bout_W, bout_b = f(inputs["bout_W"]), f(inputs["bout_b"])
    out_W, out_b = f(inputs["out_W"]), f(inputs["out_b"])
    ln1_g, ln1_b = f(inputs["ln1_g"]), f(inputs["ln1_b"])
    ln2_g, ln2_b = f(inputs["ln2_g"]), f(inputs["ln2_b"])

    ones_n = np.ones((1, N), f32)
    ones_r = np.ones((1, R), f32)
    com = {}
    com["nfT_aug"] = np.ascontiguousarray(
        np.concatenate([nf.T, ones_n], 0))
    com["amdsT_aug"] = np.ascontiguousarray(
        np.concatenate([amds.T, ones_n], 0))
    com["embW_aug"] = np.concatenate([emb_W, emb_b[None, :]], 0)
    com["bembW_aug"] = np.concatenate([bemb_W, bemb_b[None, :]], 0)
    com["qkvW_aug"] = np.ascontiguousarray(
        np.concatenate([qkv_W, qkv_b[:, None, :]], 1)).astype(NP_BF16)
    com["dWf0_aug"] = np.concatenate([diff_W[0], diff_b[0][None, :]], 0)
    com["diffW_dup"] = np.ascontiguousarray(
        np.concatenate([diff_W, diff_W], 1)).astype(NP_BF16)
    com["diffb_cols"] = np.ascontiguousarray(
        diff_b.reshape(L, H, HD).transpose(0, 2, 1))
    bwd = np.zeros((L, HD, 8 * D), f32)
    for l in range(L):
        for h in range(H):
            chunk = bout_W[l, h * HD:(h + 1) * HD, :]  # [128, 64]
            bwd[l, :, h * 2 * D:h * 2 * D + D] = chunk
            bwd[l, :, h * 2 * D + D:h * 2 * D + 2 * D] = chunk
    com["boutW_dup"] = bwd.astype(NP_BF16)
    com["boutb2"] = np.ascontiguousarray(
        np.tile(bout_b, (1, 2)).T)  # [128, L]
    com["oW"] = o_W.astype(NP_BF16)
    com["ob_cols"] = np.ascontiguousarray(o_b.T)
    com["outW_aug"] = np.concatenate([out_W, out_b[None, :]], 0)
    com["ln1g_t"] = np.tile(ln1_g[None, :], (HD, 1))
    com["ln1b_t"] = np.tile(ln1_b[None, :], (HD, 1))
    com["ln2g_t"] = np.tile(ln2_g[None, :], (HD, 1))
    com["ln2b_t"] = np.tile(ln2_b[None, :], (HD, 1))
    strip = np.zeros((HD, 255), f32)
    strip[:, 127] = 1.0
    com["strip"] = strip.astype(NP_BF16)

    in_maps = []
    for c in range(NCORES):
        m = dict(com)
        m["nfT_loc"] = np.ascontiguousarray(
            np.concatenate([nf.T[:, c * R:(c + 1) * R], ones_r], 0))
        m["amdsT_loc"] = np.ascontiguousarray(
            np.concatenate([amds.T[:, c * R:(c + 1) * R], ones_r], 0))
        in_maps.append(m)
    diffb_nonzero = bool(np.any(diff_b != 0.0))
    return in_maps, diffb_nonzero


_LAST_RESULTS = None


def kernel(**inputs) -> np.ndarray:
    global _LAST_RESULTS
    in_maps, diffb_nonzero = _prep_inputs(inputs)
    trunc = int(os.environ.get("KTRUNC", "0"))
    nc = _build(diffb_nonzero, trunc)
    trace = bool(int(os.environ.get("KERNEL_TRACE", "0")))
    try:
        res = bass_utils.run_bass_kernel_spmd(
            nc, in_maps, core_ids=list(range(NCORES)), trace=trace)
    except ModuleNotFoundError:
        res = bass_utils.run_bass_kernel_spmd(
            nc, in_maps, core_ids=list(range(NCORES)), trace=False)
    _LAST_RESULTS = res
    out = np.concatenate(
        [res.results[c]["out_loc"] for c in range(NCORES)], axis=0)
    return out.astype(np.float32)


if __name__ == "__main__":
    rng = np.random.default_rng(0)
    dummy = {
        "node_features": rng.standard_normal((N, FB), dtype=np.float32),
        "amds": rng.random((N, K), dtype=np.float32),
        "emb_W": rng.standard_normal((FB, D), dtype=np.float32) / 16,
        "emb_b": np.zeros((D,), np.float32),
        "bias_emb_W": rng.standard_normal((K, D), dtype=np.float32) / 10,
        "bias_emb_b": np.zeros((D,), np.float32),
        "ln1_g": np.ones((D,), np.float32),
        "ln1_b": np.zeros((D,), np.float32),
        "ln2_g": np.ones((D,), np.float32),
        "ln2_b": np.zeros((D,), np.float32),
        "qkv_W": rng.standard_normal((L, D, 3 * HHD), dtype=np.float32) / 8,
        "qkv_b": np.zeros((L, 3 * HHD), np.float32),
        "diff_W": rng.standard_normal((L, D, HHD), dtype=np.float32) / 8,
        "diff_b": np.zeros((L, HHD), np.float32),
        "o_W": rng.standard_normal((L, HHD, D), dtype=np.float32) / 22,
        "o_b": np.zeros((L, D), np.float32),
        "bout_W": rng.standard_normal((L, HHD, D), dtype=np.float32) / 22,
        "bout_b": np.zeros((L, D), np.float32),
        "out_W": rng.standard_normal((D, 1), dtype=np.float32) / 8,
        "out_b": np.zeros((1,), np.float32),
    }
    out = kernel(**dummy)
    print("kernel output shape:", out.shape, "first:", out[:4, 0])

